# revision 18
# baseline (speedup 1.0000x reference)
"""Trainium2 Bass kernel for the adaLN (DiT-style) dense transformer block.

Sharding: data-parallel over B — core b computes batch element b (B=8, 8 cores,
no collectives). Host-side prep is layout-only: weight transposes + bf16 casts.

Per-core dataflow (T=2048 tokens, C=512, H=8 heads, DH=64, MLP=2048):
  - LN stats + modulation in token-major (bn_stats over free dim, per-token
    scalars ride tensor_scalar per-partition operands)
  - big matmuls in feature-major (contraction dim on partitions); h is
    PE-transposed into feature-major after modulation
  - attention per head: S.T tiles [tk,tq] via lhsT=k.T, exp on ScalarE straight
    from PSUM (scale=1/8 folded in, no max-subtraction — logits are bounded),
    o via lhsT=[v|ones] so the softmax denominator rides the same matmul
  - proj/fc2 run "swapped" (lhsT=activations) so their outputs land
    token-major and the residual adds need no extra transpose
"""

import numpy as np
import ml_dtypes

import concourse.bass as bass
import concourse.bacc as bacc
import concourse.tile as tile
import concourse.mybir as mybir
from concourse.bass_utils import run_bass_kernel_spmd
from concourse.masks import make_identity

F32 = mybir.dt.float32
BF16 = mybir.dt.bfloat16
AF = mybir.ActivationFunctionType
ALU = mybir.AluOpType

B, T, C = 8, 2048, 512
H, DH, MLP = 8, 64, 4 * 512
P = 128
NT = T // P          # 16 token tiles
KC = C // P          # 4 feature chunks
NQ = T // 512        # 4 tq/tk column chunks of 512
EPS = 1e-5
GELU_AF = AF.Gelu_apprx_tanh  # test.py sim swaps to Tanh (CoreSim lacks gelu)


def build_program():
    nc = bacc.Bacc("TRN2", target_bir_lowering=False, debug=False)

    # ---- DRAM I/O ----
    x_d = nc.dram_tensor("x", [NT, P, C], F32, kind="ExternalInput").ap()
    c_col = nc.dram_tensor("c_col", [P, KC], F32, kind="ExternalInput").ap()
    ada_wt = nc.dram_tensor("ada_wt", [KC, P, 6 * C], BF16, kind="ExternalInput").ap()
    qkv_wt = nc.dram_tensor("qkv_wt", [KC, P, 3 * C], BF16, kind="ExternalInput").ap()
    proj_wt = nc.dram_tensor("proj_wt", [KC, P, C], BF16, kind="ExternalInput").ap()
    fc1_wt = nc.dram_tensor("fc1_wt", [KC, P, MLP], BF16, kind="ExternalInput").ap()
    fc2_wt = nc.dram_tensor("fc2_wt", [MLP // P, P, C], BF16, kind="ExternalInput").ap()
    qkv_b_qk = nc.dram_tensor("qkv_b_qk", [P, 8], F32, kind="ExternalInput").ap()
    fc1_b_c = nc.dram_tensor("fc1_b_c", [P, MLP // P], F32, kind="ExternalInput").ap()
    # host-folded constant rows (see make_in_maps): per branch br:
    #   A=ln_w, D=ln_w*(1+ada_b_sc), A2=ln_b, E=ln_b*(1+ada_b_sc)+ada_b_sh,
    #   pb=out-proj bias, gb=ada_b gate chunk; plus vb = qkv_b v-slice
    rows_d = {}
    for nm in (["vb_row"] +
               [f"{p}{br}" for br in (1, 2) for p in ("A", "D", "A2", "E", "pb", "gb")]):
        rows_d[nm] = nc.dram_tensor(nm, [1, C], F32, kind="ExternalInput").ap()
    out_d = nc.dram_tensor("out", [NT, P, C], F32, kind="ExternalOutput").ap()
    # DRAM bounce buffers: partition-broadcast DMA needs a DRAM source
    mod_scr = nc.dram_tensor("mod_scr", [6, C], F32).ap()
    rec_scr = nc.dram_tensor("rec_scr", [H * NQ, 512], F32).ap()

    from contextlib import ExitStack
    with tile.TileContext(nc) as tc, ExitStack() as ctx:
        consts = ctx.enter_context(tc.tile_pool(name="consts", bufs=1))
        wbig = ctx.enter_context(tc.tile_pool(name="wbig", bufs=8))
        wsmall = ctx.enter_context(tc.tile_pool(name="wsmall", bufs=16))
        bigT = ctx.enter_context(tc.tile_pool(name="bigT", bufs=8))
        qk_pool = ctx.enter_context(tc.tile_pool(name="qk", bufs=8))
        vpool = ctx.enter_context(tc.tile_pool(name="vp", bufs=NT))
        work = ctx.enter_context(tc.tile_pool(name="work", bufs=2))
        psum = ctx.enter_context(tc.tile_pool(name="ps", bufs=2, space="PSUM"))

        # ---- persistent SBUF loads ----
        sx = []
        for i in range(NT):
            t = consts.tile([P, C], F32, name=f"x{i}")
            nc.sync.dma_start(t, x_d[i])
            sx.append(t)
        ident = consts.tile([P, P], BF16, name="ident")
        make_identity(nc, ident)
        eps_t = consts.tile([P, 1], F32, name="eps_t")
        nc.gpsimd.memset(eps_t, EPS)
        sc_col = consts.tile([P, KC], F32, name="sc_col")
        nc.sync.dma_start(sc_col, c_col)
        qkvb_sb = consts.tile([P, 8], F32, name="qkvb_sb")
        nc.sync.dma_start(qkvb_sb, qkv_b_qk)
        fc1b_sb = consts.tile([P, MLP // P], F32, name="fc1b_sb")
        nc.sync.dma_start(fc1b_sb, fc1_b_c)

        # ada weights stream through wbig slots as [128, 1536] halves
        ada_sb = []
        for k in range(KC):
            halves = []
            for hh in range(2):
                w = wbig.tile([P, 3 * C], BF16, tag="wbig", name=f"ada{k}{hh}")
                nc.sync.dma_start(w, ada_wt[k][:, hh * 1536:(hh + 1) * 1536])
                halves.append(w)
            ada_sb.append(halves)

        # ---- phase 0: silu(c), mod = silu(c) @ ada_w.T + ada_b ----
        es_c = work.tile([P, KC], F32, tag="esc")
        nc.scalar.activation(es_c, sc_col, AF.Exp, scale=-1.0)
        nc.vector.tensor_scalar_add(es_c, es_c, 1.0)
        nc.vector.reciprocal(es_c, es_c)
        silu_f = work.tile([P, KC], F32, tag="siluf")
        nc.vector.tensor_mul(silu_f, sc_col, es_c)
        silu_b = consts.tile([P, KC], BF16, name="silu_b")
        nc.vector.tensor_copy(silu_b, silu_f)

        def bcast(dst, src_row):
            src = bass.AP(tensor=src_row.tensor, offset=src_row.offset,
                          ap=[[0, dst.shape[0]]] + list(src_row.ap[1:]))
            nc.gpsimd.dma_start(out=dst, in_=src)

        def ada_mm_row(j):
            """mod chunk j (pre-ada_b) as a [1, C] PSUM row.
            chunks: 0=sh_msa 1=sc_msa 2=g_msa 3=sh_mlp 4=sc_mlp 5=g_mlp"""
            ps = psum.tile([P, 1024], F32, tag="mm1024", name=f"adaps{j}")
            for k in range(KC):
                hh, off = divmod(j * C, 1536)
                nc.tensor.matmul(ps[0:1, 0:C], silu_b[:, k:k + 1],
                                 ada_sb[k][hh][:, off:off + C],
                                 start=(k == 0), stop=(k == KC - 1))
            mrow = work.tile([1, C], F32, tag="mrow", bufs=2, name=f"mrow{j}")
            nc.vector.tensor_copy(mrow, ps[0:1, 0:C])
            nc.sync.dma_start(mod_scr[j:j + 1, :], mrow)
            return mod_scr[j:j + 1, :]

        def tmp_bc(src_row, nm):
            t = work.tile([P, C], F32, tag="tmp", bufs=3, name=nm)
            bcast(t, src_row)
            return t

        # modulation vectors, replicated [P, C] bf16:
        #   W = ln_w*(1+sc) = sc_dev*A + D     B = ln_b*(1+sc)+sh = sc_dev*A2 + sh_dev + E
        #   G = g_dev + gb                     GPB = G*pb
        # where *_dev are the device-computed silu(c)@ada_wT chunks.
        vecs = {}
        for br in (1, 2):
            base = (br - 1) * 3
            g_bc = tmp_bc(ada_mm_row(base + 2), f"gbc{br}")
            gb_bc = tmp_bc(rows_d[f"gb{br}"], f"gbbc{br}")
            G = consts.tile([P, C], BF16, name=f"G{br}")
            nc.vector.tensor_add(G, g_bc, gb_bc)
            pb_bc = tmp_bc(rows_d[f"pb{br}"], f"pbbc{br}")
            GPB = consts.tile([P, C], BF16, name=f"GPB{br}")
            nc.vector.tensor_mul(GPB, G, pb_bc)
            A_bc = tmp_bc(rows_d[f"A{br}"], f"abc{br}")
            D_bc = tmp_bc(rows_d[f"D{br}"], f"dbc{br}")
            sc_bc = tmp_bc(ada_mm_row(base + 1), f"scbc{br}")
            W = consts.tile([P, C], BF16, name=f"W{br}")
            nc.vector.tensor_mul(W, sc_bc, A_bc)
            nc.vector.tensor_add(W, W, D_bc)
            sh_bc = tmp_bc(ada_mm_row(base + 0), f"shbc{br}")
            A2_bc = tmp_bc(rows_d[f"A2{br}"], f"a2bc{br}")
            Bv = consts.tile([P, C], BF16, name=f"B{br}")
            nc.vector.tensor_mul(Bv, sc_bc, A2_bc)
            nc.vector.tensor_add(Bv, Bv, sh_bc)
            E_bc = tmp_bc(rows_d[f"E{br}"], f"ebc{br}")
            nc.vector.tensor_add(Bv, Bv, E_bc)
            vecs[br] = (W, Bv, G, GPB)
        (W1, B1, G1, GPB1), (W2, B2, G2, GPB2) = vecs[1], vecs[2]
        VB = consts.tile([P, C], BF16, name="VB")
        vb_bc = tmp_bc(rows_d["vb_row"], "vbbc")
        nc.vector.tensor_copy(VB, vb_bc)

        # remaining weights (wbig slots 9-16 evict ada after its matmuls)
        qkv_sb = []
        for k in range(KC):
            w = wbig.tile([P, 3 * C], BF16, tag="wbig", name=f"qkvw{k}")
            nc.sync.dma_start(w, qkv_wt[k])
            qkv_sb.append(w)
        fc1_sb = []
        for k in range(KC):
            w = wbig.tile([P, MLP], BF16, tag="wbig", name=f"fc1w{k}")
            nc.sync.dma_start(w, fc1_wt[k])
            fc1_sb.append(w)
        proj_sb = []
        for k in range(KC):
            w = wbig.tile([P, C], BF16, tag="wbig", name=f"projw{k}")
            nc.sync.dma_start(w, proj_wt[k])
            proj_sb.append(w)
        fc2_sb = []
        for k in range(MLP // P):
            w = wsmall.tile([P, C], BF16, tag="wsmall", name=f"fc2w{k}")
            nc.sync.dma_start(w, fc2_wt[k])
            fc2_sb.append(w)

        # ---- LN + modulate + transpose to feature-major ----
        def ln_modulate(xt, i, Wt, Bt, hT, stats_tag):
            st = work.tile([P, 6], F32, tag="st", bufs=2, name=f"st{stats_tag}{i}")
            nc.vector.bn_stats(st, xt)
            mv = work.tile([P, 2], F32, tag="mv", bufs=2, name=f"mv{stats_tag}{i}")
            nc.vector.bn_aggr(mv, st)
            rstd = work.tile([P, 1], F32, tag="rstd", bufs=2, name=f"rstd{stats_tag}{i}")
            nc.scalar.activation(rstd, mv[:, 1:2], AF.Ln, bias=eps_t)
            nc.scalar.activation(rstd, rstd, AF.Exp, scale=-0.5)
            t1 = work.tile([P, C], F32, tag="t1", bufs=2, name=f"t1{stats_tag}{i}")
            nc.vector.tensor_scalar(t1, xt, mv[:, 0:1], rstd,
                                    op0=ALU.subtract, op1=ALU.mult)
            nc.vector.tensor_mul(t1, t1, Wt)
            hb = work.tile([P, C], BF16, tag="hb", bufs=2, name=f"hb{stats_tag}{i}")
            nc.vector.tensor_add(hb, t1, Bt)
            for j in range(KC):
                tp = psum.tile([P, P], BF16, tag="mm1024", name=f"tp{stats_tag}_{i}_{j}")
                nc.tensor.transpose(tp, hb[:, j * P:(j + 1) * P], ident)
                nc.vector.tensor_copy(hT[j][:, i * P:(i + 1) * P], tp)

        h1T = [bigT.tile([P, T], BF16, tag="bigT", name=f"h1T{j}") for j in range(KC)]
        for i in range(NT):
            ln_modulate(sx[i], i, W1, B1, h1T, "a")

        # ---- qkv: q,k feature-major [8 x (P, T)]; v token-major interleaved ----
        qkT = [qk_pool.tile([P, T], BF16, tag="qk", name=f"qkT{m}") for m in range(8)]
        for m in range(8):
            pss = [psum.tile([P, C], F32, tag="oacc", bufs=4, name=f"qkps{m}_{n}")
                   for n in range(NQ)]
            for k in range(KC):
                for n in range(NQ):
                    nc.tensor.matmul(pss[n], qkv_sb[k][:, m * P:(m + 1) * P],
                                     h1T[k][:, n * 512:(n + 1) * 512],
                                     start=(k == 0), stop=(k == KC - 1))
            for n in range(NQ):
                nc.vector.tensor_scalar_add(qkT[m][:, n * 512:(n + 1) * 512],
                                            pss[n], qkvb_sb[:, m:m + 1])

        # v: out token-major [t, c_v], scattered into [128, 8, 65] (| ones)
        vtok = [vpool.tile([P, H * 65], BF16, tag="vtok", name=f"vtok{i}")
                for i in range(NT)]
        for i in range(NT):
            ps = psum.tile([P, 1024], F32, tag="mm1024", name=f"vps{i}")
            for k in range(KC):
                nc.tensor.matmul(ps[:, 0:C], h1T[k][:, i * P:(i + 1) * P],
                                 qkv_sb[k][:, 2 * C:3 * C],
                                 start=(k == 0), stop=(k == KC - 1))
            src = ps[:, 0:C].rearrange("p (h d) -> p h d", h=H)
            dst3 = vtok[i].rearrange("p (h d) -> p h d", d=65)[:, :, 0:DH]
            vb3 = VB.rearrange("p (h d) -> p h d", h=H)
            nc.vector.tensor_add(dst3, src, vb3)
            ones_col = vtok[i].rearrange("p (h d) -> p h d", d=65)[:, :, DH:65]
            nc.gpsimd.memset(ones_col, 1.0)

        # ---- attention ----
        oT = [bigT.tile([P, T], BF16, tag="bigT", name=f"oT{j}") for j in range(KC)]
        rc_pool = ctx.enter_context(tc.tile_pool(name="rc", bufs=2))
        for h in range(H):
            qh = qkT[h // 2][(h % 2) * DH:(h % 2) * DH + DH, :]
            kh = qkT[4 + h // 2][(h % 2) * DH:(h % 2) * DH + DH, :]
            oacc = [psum.tile([P, C], F32, tag="oacc", bufs=4, name=f"oacc{h}_{n}")
                    for n in range(NQ)]
            for tk in range(NT):
                vsl = vtok[tk][:, h * 65:h * 65 + 65]
                for g in range(2):
                    sg = psum.tile([P, 1024], F32, tag="mm1024", name=f"sg{h}_{tk}_{g}")
                    for n2 in range(2):
                        n = 2 * g + n2
                        nc.tensor.matmul(sg[:, n2 * 512:(n2 + 1) * 512],
                                         kh[:, tk * P:(tk + 1) * P],
                                         qh[:, n * 512:(n + 1) * 512],
                                         start=True, stop=True)
                    es = work.tile([P, 1024], BF16, tag="es", bufs=2,
                                   name=f"es{h}_{tk}_{g}")
                    nc.scalar.activation(es, sg, AF.Exp, scale=0.125)
                    for n2 in range(2):
                        n = 2 * g + n2
                        nc.tensor.matmul(oacc[n][0:65, :], vsl,
                                         es[:, n2 * 512:(n2 + 1) * 512],
                                         start=(tk == 0), stop=(tk == NT - 1))
            for n in range(NQ):
                rrow = rc_pool.tile([1, 512], F32, tag="rrow", bufs=2,
                                    name=f"rr{h}_{n}")
                nc.vector.reciprocal(rrow, oacc[n][DH:DH + 1, :])
                ridx = h * NQ + n
                nc.sync.dma_start(rec_scr[ridx:ridx + 1, :], rrow)
                rbc = rc_pool.tile([DH, 512], F32, tag="rbc", bufs=2,
                                   name=f"rb{h}_{n}")
                bcast(rbc, rec_scr[ridx:ridx + 1, :])
                nc.vector.tensor_mul(
                    oT[h // 2][(h % 2) * DH:(h % 2) * DH + DH,
                               n * 512:(n + 1) * 512],
                    oacc[n][0:DH, :], rbc)

        # ---- proj (swapped: token-major out) + residual 1 (in-place x) ----
        for i in range(NT):
            ps = psum.tile([P, 1024], F32, tag="mm1024", name=f"prps{i}")
            for k in range(KC):
                nc.tensor.matmul(ps[:, 0:C], oT[k][:, i * P:(i + 1) * P],
                                 proj_sb[k], start=(k == 0), stop=(k == KC - 1))
            ta = work.tile([P, C], F32, tag="tmp", bufs=3, name=f"res1_{i}")
            nc.vector.tensor_mul(ta, ps[:, 0:C], G1)
            nc.vector.tensor_add(ta, ta, GPB1)
            nc.vector.tensor_add(sx[i], sx[i], ta)

        # ---- LN2 + modulate + transpose (h2T reuses h1T slots) ----
        h2T = [bigT.tile([P, T], BF16, tag="bigT", name=f"h2T{j}") for j in range(KC)]
        for i in range(NT):
            ln_modulate(sx[i], i, W2, B2, h2T, "b")

        # ---- MLP per t-chunk; fc2 swapped -> token-major; residual 2 ----
        for n in range(NQ):
            fps = [psum.tile([P, C], F32, tag="oacc", bufs=4, name=f"fps{n}_{s}")
                   for s in range(4)]
            for m in range(MLP // P):
                ps = psum.tile([P, 1024], F32, tag="mm1024", name=f"f1ps{n}_{m}")
                for k in range(KC):
                    nc.tensor.matmul(ps[:, 0:C], fc1_sb[k][:, m * P:(m + 1) * P],
                                     h2T[k][:, n * 512:(n + 1) * 512],
                                     start=(k == 0), stop=(k == KC - 1))
                g1 = work.tile([P, C], BF16, tag="g1", bufs=2, name=f"g1_{n}_{m}")
                nc.scalar.activation(g1, ps[:, 0:C], GELU_AF,
                                     bias=fc1b_sb[:, m:m + 1])
                for s in range(4):
                    nc.tensor.matmul(fps[s], g1[:, s * P:(s + 1) * P], fc2_sb[m],
                                     start=(m == 0), stop=(m == MLP // P - 1))
            for s in range(4):
                i = n * 4 + s
                tb = work.tile([P, C], F32, tag="tmp", bufs=3, name=f"res2_{i}")
                nc.vector.tensor_mul(tb, fps[s], G2)
                nc.vector.tensor_add(tb, tb, GPB2)
                nc.vector.tensor_add(sx[i], sx[i], tb)
                nc.sync.dma_start(out_d[i], sx[i])

    nc.compile()
    return nc


def make_in_maps(inputs):
    bf = ml_dtypes.bfloat16
    f32 = np.float32
    x = np.asarray(inputs["x"], f32)
    c = np.asarray(inputs["c"], f32)
    qkv_w = np.asarray(inputs["qkv_w"], f32)
    qkv_b = np.asarray(inputs["qkv_b"], f32)
    proj_w = np.asarray(inputs["proj_w"], f32)
    proj_b = np.asarray(inputs["proj_b"], f32)
    ada_w = np.asarray(inputs["ada_w"], f32)
    ada_b = np.asarray(inputs["ada_b"], f32)
    fc1_w = np.asarray(inputs["fc1_w"], f32)
    fc1_b = np.asarray(inputs["fc1_b"], f32)
    fc2_w = np.asarray(inputs["fc2_w"], f32)
    fc2_b = np.asarray(inputs["fc2_b"], f32)
    ln = {k: np.asarray(inputs[k], f32) for k in
          ["ln1_w", "ln1_b", "ln2_w", "ln2_b"]}

    shared = {
        "ada_wt": np.ascontiguousarray(ada_w.T.reshape(KC, P, 6 * C)).astype(bf),
        "qkv_wt": np.ascontiguousarray(qkv_w.T.reshape(KC, P, 3 * C)).astype(bf),
        "proj_wt": np.ascontiguousarray(proj_w.T.reshape(KC, P, C)).astype(bf),
        "fc1_wt": np.ascontiguousarray(fc1_w.T.reshape(KC, P, MLP)).astype(bf),
        "fc2_wt": np.ascontiguousarray(fc2_w.T.reshape(MLP // P, P, C)).astype(bf),
        "qkv_b_qk": np.ascontiguousarray(qkv_b[:2 * C].reshape(8, P).T).astype(f32),
        "fc1_b_c": np.ascontiguousarray(fc1_b.reshape(MLP // P, P).T).astype(f32),
        "vb_row": qkv_b[2 * C:].reshape(1, C).astype(f32),
    }
    # host-folded constant rows (weights-only algebra; inputs never touched):
    #   W = ln_w*(1+mod_sc) where mod_sc = dev_sc + ada_b_sc
    #     = dev_sc*A + D with A = ln_w, D = ln_w*(1+ada_b_sc); similarly B, G.
    for br, (lnw, lnb, pb) in {1: (ln["ln1_w"], ln["ln1_b"], proj_b),
                               2: (ln["ln2_w"], ln["ln2_b"], fc2_b)}.items():
        o = (br - 1) * 3 * C
        sh_ab = ada_b[o:o + C]
        sc_ab = ada_b[o + C:o + 2 * C]
        g_ab = ada_b[o + 2 * C:o + 3 * C]
        shared[f"A{br}"] = lnw.reshape(1, C).astype(f32)
        shared[f"D{br}"] = (lnw * (1 + sc_ab)).reshape(1, C).astype(f32)
        shared[f"A2{br}"] = lnb.reshape(1, C).astype(f32)
        shared[f"E{br}"] = (lnb * (1 + sc_ab) + sh_ab).reshape(1, C).astype(f32)
        shared[f"pb{br}"] = pb.reshape(1, C).astype(f32)
        shared[f"gb{br}"] = g_ab.reshape(1, C).astype(f32)
    maps = []
    for b in range(B):
        m = dict(shared)
        m["x"] = np.ascontiguousarray(x[b].reshape(NT, P, C))
        m["c_col"] = np.ascontiguousarray(c[b].reshape(KC, P).T)
        maps.append(m)
    return maps


_CACHED_NC = None


def run(inputs, trace=False):
    global _CACHED_NC
    if _CACHED_NC is None:
        _CACHED_NC = build_program()
    maps = make_in_maps(inputs)
    res = run_bass_kernel_spmd(_CACHED_NC, maps, core_ids=list(range(B)),
                               trace=trace)
    out = np.stack([res.results[b]["out"].reshape(T, C) for b in range(B)])
    return out.astype(np.float32), res


def kernel(**inputs) -> np.ndarray:
    out, _ = run(inputs, trace=False)
    return out


# revision 19
# speedup vs baseline: 1.2329x; 1.2329x over previous
"""Trainium2 Bass kernel for the adaLN (DiT-style) dense transformer block.

Sharding: data-parallel over B — core b computes batch element b (B=8, 8 cores,
no collectives). Host-side prep is layout-only: weight transposes + bf16 casts.

Per-core dataflow (T=2048 tokens, C=512, H=8 heads, DH=64, MLP=2048):
  - LN stats + modulation in token-major (bn_stats over free dim, per-token
    scalars ride tensor_scalar per-partition operands)
  - big matmuls in feature-major (contraction dim on partitions); h is
    PE-transposed into feature-major after modulation
  - attention per head: S.T tiles [tk,tq] via lhsT=k.T, exp on ScalarE straight
    from PSUM (scale=1/8 folded in, no max-subtraction — logits are bounded),
    o via lhsT=[v|ones] so the softmax denominator rides the same matmul
  - proj/fc2 run "swapped" (lhsT=activations) so their outputs land
    token-major and the residual adds need no extra transpose
"""

import numpy as np
import ml_dtypes

import concourse.bass as bass
import concourse.bacc as bacc
import concourse.tile as tile
import concourse.mybir as mybir
from concourse.bass_utils import run_bass_kernel_spmd
from concourse.masks import make_identity

F32 = mybir.dt.float32
BF16 = mybir.dt.bfloat16
AF = mybir.ActivationFunctionType
ALU = mybir.AluOpType

B, T, C = 8, 2048, 512
H, DH, MLP = 8, 64, 4 * 512
P = 128
NT = T // P          # 16 token tiles
KC = C // P          # 4 feature chunks
NQ = T // 512        # 4 tq/tk column chunks of 512
EPS = 1e-5
GELU_AF = AF.Gelu_apprx_tanh  # test.py sim swaps to Tanh (CoreSim lacks gelu)


def build_program():
    nc = bacc.Bacc("TRN2", target_bir_lowering=False, debug=False)

    # ---- DRAM I/O ----
    x_d = nc.dram_tensor("x", [NT, P, C], F32, kind="ExternalInput").ap()
    c_col = nc.dram_tensor("c_col", [P, KC], F32, kind="ExternalInput").ap()
    ada_wt = nc.dram_tensor("ada_wt", [KC, P, 6 * C], BF16, kind="ExternalInput").ap()
    qkv_wt = nc.dram_tensor("qkv_wt", [KC, P, 3 * C], BF16, kind="ExternalInput").ap()
    proj_wt = nc.dram_tensor("proj_wt", [KC, P, C], BF16, kind="ExternalInput").ap()
    fc1_wt = nc.dram_tensor("fc1_wt", [KC, P, MLP], BF16, kind="ExternalInput").ap()
    fc2_wt = nc.dram_tensor("fc2_wt", [MLP // P, P, C], BF16, kind="ExternalInput").ap()
    qkv_b_qk = nc.dram_tensor("qkv_b_qk", [P, 8], F32, kind="ExternalInput").ap()
    fc1_b_c = nc.dram_tensor("fc1_b_c", [P, MLP // P], F32, kind="ExternalInput").ap()
    # host-folded constant rows (see make_in_maps): per branch br:
    #   A=ln_w, D=ln_w*(1+ada_b_sc), A2=ln_b, E=ln_b*(1+ada_b_sc)+ada_b_sh,
    #   pb=out-proj bias, gb=ada_b gate chunk; plus vb = qkv_b v-slice
    rows_d = {}
    for nm in (["vb_row"] +
               [f"{p}{br}" for br in (1, 2) for p in ("A", "D", "A2", "E", "pb", "gb")]):
        rows_d[nm] = nc.dram_tensor(nm, [1, C], F32, kind="ExternalInput").ap()
    out_d = nc.dram_tensor("out", [NT, P, C], F32, kind="ExternalOutput").ap()
    # DRAM bounce buffers: partition-broadcast DMA needs a DRAM source
    mod_scr = nc.dram_tensor("mod_scr", [6, C], F32).ap()
    rec_scr = nc.dram_tensor("rec_scr", [H * NQ, 512], F32).ap()

    from contextlib import ExitStack
    with tile.TileContext(nc) as tc, ExitStack() as ctx:
        consts = ctx.enter_context(tc.tile_pool(name="consts", bufs=1))
        wbig = ctx.enter_context(tc.tile_pool(name="wbig", bufs=8))
        wsmall = ctx.enter_context(tc.tile_pool(name="wsmall", bufs=16))
        bigT = ctx.enter_context(tc.tile_pool(name="bigT", bufs=8))
        qk_pool = ctx.enter_context(tc.tile_pool(name="qk", bufs=8))
        vpool = ctx.enter_context(tc.tile_pool(name="vp", bufs=NT))
        work = ctx.enter_context(tc.tile_pool(name="work", bufs=2))
        psum = ctx.enter_context(tc.tile_pool(name="ps", bufs=2, space="PSUM"))

        # ---- persistent SBUF loads ----
        sx = []
        for i in range(NT):
            t = consts.tile([P, C], F32, name=f"x{i}")
            nc.sync.dma_start(t, x_d[i])
            sx.append(t)
        ident = consts.tile([P, P], BF16, name="ident")
        make_identity(nc, ident)
        eps_t = consts.tile([P, 1], F32, name="eps_t")
        nc.gpsimd.memset(eps_t, EPS)
        sc_col = consts.tile([P, KC], F32, name="sc_col")
        nc.sync.dma_start(sc_col, c_col)
        qkvb_sb = consts.tile([P, 8], F32, name="qkvb_sb")
        nc.sync.dma_start(qkvb_sb, qkv_b_qk)
        fc1b_sb = consts.tile([P, MLP // P], F32, name="fc1b_sb")
        nc.sync.dma_start(fc1b_sb, fc1_b_c)

        # ada weights stream through wbig slots as [128, 1536] halves
        ada_sb = []
        for k in range(KC):
            halves = []
            for hh in range(2):
                w = wbig.tile([P, 3 * C], BF16, tag="wbig", name=f"ada{k}{hh}")
                nc.sync.dma_start(w, ada_wt[k][:, hh * 1536:(hh + 1) * 1536])
                halves.append(w)
            ada_sb.append(halves)

        # ---- phase 0: silu(c), mod = silu(c) @ ada_w.T + ada_b ----
        es_c = work.tile([P, KC], F32, tag="esc")
        nc.scalar.activation(es_c, sc_col, AF.Exp, scale=-1.0)
        nc.vector.tensor_scalar_add(es_c, es_c, 1.0)
        nc.vector.reciprocal(es_c, es_c)
        silu_f = work.tile([P, KC], F32, tag="siluf")
        nc.vector.tensor_mul(silu_f, sc_col, es_c)
        silu_b = consts.tile([P, KC], BF16, name="silu_b")
        nc.vector.tensor_copy(silu_b, silu_f)

        def bcast(dst, src_row):
            src = bass.AP(tensor=src_row.tensor, offset=src_row.offset,
                          ap=[[0, dst.shape[0]]] + list(src_row.ap[1:]))
            nc.gpsimd.dma_start(out=dst, in_=src)

        def ada_mm_row(j):
            """mod chunk j (pre-ada_b) as a [1, C] PSUM row.
            chunks: 0=sh_msa 1=sc_msa 2=g_msa 3=sh_mlp 4=sc_mlp 5=g_mlp"""
            ps = psum.tile([P, 1024], F32, tag="sg", name=f"adaps{j}")
            for k in range(KC):
                hh, off = divmod(j * C, 1536)
                nc.tensor.matmul(ps[0:1, 0:C], silu_b[:, k:k + 1],
                                 ada_sb[k][hh][:, off:off + C],
                                 start=(k == 0), stop=(k == KC - 1))
            mrow = work.tile([1, C], F32, tag="mrow", bufs=2, name=f"mrow{j}")
            nc.vector.tensor_copy(mrow, ps[0:1, 0:C])
            nc.sync.dma_start(mod_scr[j:j + 1, :], mrow)
            return mod_scr[j:j + 1, :]

        def tmp_bc(src_row, nm):
            t = work.tile([P, C], F32, tag="tmp", bufs=3, name=nm)
            bcast(t, src_row)
            return t

        # modulation vectors, replicated [P, C] bf16:
        #   W = ln_w*(1+sc) = sc_dev*A + D     B = ln_b*(1+sc)+sh = sc_dev*A2 + sh_dev + E
        #   G = g_dev + gb                     GPB = G*pb
        # where *_dev are the device-computed silu(c)@ada_wT chunks.
        vecs = {}
        for br in (1, 2):
            base = (br - 1) * 3
            g_bc = tmp_bc(ada_mm_row(base + 2), f"gbc{br}")
            gb_bc = tmp_bc(rows_d[f"gb{br}"], f"gbbc{br}")
            G = consts.tile([P, C], BF16, name=f"G{br}")
            nc.vector.tensor_add(G, g_bc, gb_bc)
            pb_bc = tmp_bc(rows_d[f"pb{br}"], f"pbbc{br}")
            GPB = consts.tile([P, C], BF16, name=f"GPB{br}")
            nc.vector.tensor_mul(GPB, G, pb_bc)
            A_bc = tmp_bc(rows_d[f"A{br}"], f"abc{br}")
            D_bc = tmp_bc(rows_d[f"D{br}"], f"dbc{br}")
            sc_bc = tmp_bc(ada_mm_row(base + 1), f"scbc{br}")
            W = consts.tile([P, C], BF16, name=f"W{br}")
            nc.vector.tensor_mul(W, sc_bc, A_bc)
            nc.vector.tensor_add(W, W, D_bc)
            sh_bc = tmp_bc(ada_mm_row(base + 0), f"shbc{br}")
            A2_bc = tmp_bc(rows_d[f"A2{br}"], f"a2bc{br}")
            Bv = consts.tile([P, C], BF16, name=f"B{br}")
            nc.vector.tensor_mul(Bv, sc_bc, A2_bc)
            nc.vector.tensor_add(Bv, Bv, sh_bc)
            E_bc = tmp_bc(rows_d[f"E{br}"], f"ebc{br}")
            nc.vector.tensor_add(Bv, Bv, E_bc)
            vecs[br] = (W, Bv, G, GPB)
        (W1, B1, G1, GPB1), (W2, B2, G2, GPB2) = vecs[1], vecs[2]
        VB = consts.tile([P, C], BF16, name="VB")
        vb_bc = tmp_bc(rows_d["vb_row"], "vbbc")
        nc.vector.tensor_copy(VB, vb_bc)

        # remaining weights (wbig slots 9-16 evict ada after its matmuls)
        qkv_sb = []
        for k in range(KC):
            w = wbig.tile([P, 3 * C], BF16, tag="wbig", name=f"qkvw{k}")
            nc.sync.dma_start(w, qkv_wt[k])
            qkv_sb.append(w)
        fc1_sb = []
        for k in range(KC):
            w = wbig.tile([P, MLP], BF16, tag="wbig", name=f"fc1w{k}")
            nc.sync.dma_start(w, fc1_wt[k])
            fc1_sb.append(w)
        proj_sb = []
        for k in range(KC):
            w = wbig.tile([P, C], BF16, tag="wbig", name=f"projw{k}")
            nc.sync.dma_start(w, proj_wt[k])
            proj_sb.append(w)
        fc2_sb = []
        for k in range(MLP // P):
            w = wsmall.tile([P, C], BF16, tag="wsmall", name=f"fc2w{k}")
            nc.sync.dma_start(w, fc2_wt[k])
            fc2_sb.append(w)

        # ---- LN + modulate + transpose to feature-major ----
        def ln_modulate(xt, i, Wt, Bt, hT, stats_tag):
            st = work.tile([P, 6], F32, tag="st", bufs=2, name=f"st{stats_tag}{i}")
            nc.vector.bn_stats(st, xt)
            mv = work.tile([P, 2], F32, tag="mv", bufs=2, name=f"mv{stats_tag}{i}")
            nc.vector.bn_aggr(mv, st)
            rstd = work.tile([P, 1], F32, tag="rstd", bufs=2, name=f"rstd{stats_tag}{i}")
            nc.scalar.activation(rstd, mv[:, 1:2], AF.Ln, bias=eps_t)
            nc.scalar.activation(rstd, rstd, AF.Exp, scale=-0.5)
            negmr = work.tile([P, 1], F32, tag="negmr", bufs=2,
                              name=f"negmr{stats_tag}{i}")
            nc.vector.tensor_scalar(negmr, mv[:, 0:1], rstd, -1.0,
                                    op0=ALU.mult, op1=ALU.mult)
            t1 = work.tile([P, C], F32, tag="t1", bufs=2, name=f"t1{stats_tag}{i}")
            nc.scalar.activation(t1, xt, AF.Identity, bias=negmr, scale=rstd)
            nc.vector.tensor_mul(t1, t1, Wt)
            hb = work.tile([P, C], BF16, tag="hb", bufs=2, name=f"hb{stats_tag}{i}")
            nc.vector.tensor_add(hb, t1, Bt)
            for j in range(KC):
                tp = psum.tile([P, P], BF16, tag="sg", name=f"tp{stats_tag}_{i}_{j}")
                nc.tensor.transpose(tp, hb[:, j * P:(j + 1) * P], ident)
                nc.any.tensor_copy(hT[j][:, i * P:(i + 1) * P], tp)

        h1T = [bigT.tile([P, T], BF16, tag="bigT", name=f"h1T{j}") for j in range(KC)]
        for i in range(NT):
            ln_modulate(sx[i], i, W1, B1, h1T, "a")

        # ---- qkv: q,k feature-major [8 x (P, T)]; v token-major interleaved ----
        qkT = [qk_pool.tile([P, T], BF16, tag="qk", name=f"qkT{m}") for m in range(8)]
        for m in range(8):
            prs = [psum.tile([P, 1024], F32, tag="oaccp", name=f"qkps{m}_{pp}")
                   for pp in range(2)]
            for k in range(KC):
                for n in range(NQ):
                    nc.tensor.matmul(prs[n // 2][:, (n % 2) * 512:(n % 2) * 512 + 512],
                                     qkv_sb[k][:, m * P:(m + 1) * P],
                                     h1T[k][:, n * 512:(n + 1) * 512],
                                     start=(k == 0), stop=(k == KC - 1))
            for pp in range(2):
                nc.scalar.activation(qkT[m][:, pp * 1024:(pp + 1) * 1024],
                                     prs[pp], AF.Identity,
                                     bias=qkvb_sb[:, m:m + 1])

        # v: out token-major [t, c_v], scattered into [128, 8, 65] (| ones)
        vtok = [vpool.tile([P, H * 65], BF16, tag="vtok", name=f"vtok{i}")
                for i in range(NT)]
        for i in range(NT):
            ps = psum.tile([P, 1024], F32, tag="sg", name=f"vps{i}")
            for k in range(KC):
                nc.tensor.matmul(ps[:, 0:C], h1T[k][:, i * P:(i + 1) * P],
                                 qkv_sb[k][:, 2 * C:3 * C],
                                 start=(k == 0), stop=(k == KC - 1))
            src = ps[:, 0:C].rearrange("p (h d) -> p h d", h=H)
            dst3 = vtok[i].rearrange("p (h d) -> p h d", d=65)[:, :, 0:DH]
            vb3 = VB.rearrange("p (h d) -> p h d", h=H)
            nc.vector.tensor_add(dst3, src, vb3)
            ones_col = vtok[i].rearrange("p (h d) -> p h d", d=65)[:, :, DH:65]
            nc.gpsimd.memset(ones_col, 1.0)

        # ---- attention ----
        oT = [bigT.tile([P, T], BF16, tag="bigT", name=f"oT{j}") for j in range(KC)]
        rc_pool = ctx.enter_context(tc.tile_pool(name="rc", bufs=2))
        for h in range(H):
            qh = qkT[h // 2][(h % 2) * DH:(h % 2) * DH + DH, :]
            kh = qkT[4 + h // 2][(h % 2) * DH:(h % 2) * DH + DH, :]
            for npair in range(2):
                oaccp = psum.tile([P, 1024], F32, tag="oaccp",
                                  name=f"oaccp{h}_{npair}")
                es_prev = None
                for tk in range(NT):
                    vsl = vtok[tk][:, h * 65:h * 65 + 65]
                    sg = psum.tile([P, 1024], F32, tag="sg", name=f"sg{h}_{npair}_{tk}")
                    for n2 in range(2):
                        n = 2 * npair + n2
                        nc.tensor.matmul(sg[:, n2 * 512:(n2 + 1) * 512],
                                         kh[:, tk * P:(tk + 1) * P],
                                         qh[:, n * 512:(n + 1) * 512],
                                         start=True, stop=True)
                    # o-matmuls run one tk behind so the in-order PE queue
                    # never waits on the exp of the current tk
                    if es_prev is not None:
                        vprev = vtok[tk - 1][:, h * 65:h * 65 + 65]
                        for n2 in range(2):
                            nc.tensor.matmul(
                                oaccp[0:65, n2 * 512:(n2 + 1) * 512], vprev,
                                es_prev[:, n2 * 512:(n2 + 1) * 512],
                                start=(tk - 1 == 0), stop=False)
                    es = work.tile([P, 1024], BF16, tag="es", bufs=3,
                                   name=f"es{h}_{npair}_{tk}")
                    nc.scalar.activation(es, sg, AF.Exp, scale=0.125)
                    es_prev = es
                vlast = vtok[NT - 1][:, h * 65:h * 65 + 65]
                for n2 in range(2):
                    nc.tensor.matmul(oaccp[0:65, n2 * 512:(n2 + 1) * 512], vlast,
                                     es_prev[:, n2 * 512:(n2 + 1) * 512],
                                     start=False, stop=True)
                for n2 in range(2):
                    n = 2 * npair + n2
                    osl = oaccp[:, n2 * 512:(n2 + 1) * 512]
                    rrow = rc_pool.tile([1, 512], F32, tag="rrow", bufs=2,
                                        name=f"rr{h}_{n}")
                    nc.vector.reciprocal(rrow, osl[DH:DH + 1, :])
                    ridx = h * NQ + n
                    nc.sync.dma_start(rec_scr[ridx:ridx + 1, :], rrow)
                    rbc = rc_pool.tile([DH, 512], F32, tag="rbc", bufs=2,
                                       name=f"rb{h}_{n}")
                    bcast(rbc, rec_scr[ridx:ridx + 1, :])
                    nc.vector.tensor_mul(
                        oT[h // 2][(h % 2) * DH:(h % 2) * DH + DH,
                                   n * 512:(n + 1) * 512],
                        osl[0:DH, :], rbc)

        # ---- proj (swapped: token-major out) + residual 1 (in-place x) ----
        for i in range(NT):
            ps = psum.tile([P, 1024], F32, tag="sg", name=f"prps{i}")
            for k in range(KC):
                nc.tensor.matmul(ps[:, 0:C], oT[k][:, i * P:(i + 1) * P],
                                 proj_sb[k], start=(k == 0), stop=(k == KC - 1))
            ta = work.tile([P, C], F32, tag="tmp", bufs=3, name=f"res1_{i}")
            nc.vector.tensor_mul(ta, ps[:, 0:C], G1)
            nc.vector.tensor_add(ta, ta, GPB1)
            nc.vector.tensor_add(sx[i], sx[i], ta)

        # ---- LN2 + modulate + transpose (h2T reuses h1T slots) ----
        h2T = [bigT.tile([P, T], BF16, tag="bigT", name=f"h2T{j}") for j in range(KC)]
        for i in range(NT):
            ln_modulate(sx[i], i, W2, B2, h2T, "b")

        # ---- MLP per t-chunk; fc2 swapped -> token-major; residual 2 ----
        for n in range(NQ):
            fps = [psum.tile([P, 1024], F32, tag="oaccp", name=f"fps{n}_{sp}")
                   for sp in range(2)]

            def fc2_mms(m, g1t):
                for s in range(4):
                    nc.tensor.matmul(fps[s // 2][:, (s % 2) * 512:(s % 2) * 512 + 512],
                                     g1t[:, s * P:(s + 1) * P], fc2_sb[m],
                                     start=(m == 0), stop=(m == MLP // P - 1))

            g1_prev = None
            for m in range(MLP // P):
                ps = psum.tile([P, 1024], F32, tag="sg", name=f"f1ps{n}_{m}")
                for k in range(KC):
                    nc.tensor.matmul(ps[:, 0:C], fc1_sb[k][:, m * P:(m + 1) * P],
                                     h2T[k][:, n * 512:(n + 1) * 512],
                                     start=(k == 0), stop=(k == KC - 1))
                if g1_prev is not None:
                    fc2_mms(m - 1, g1_prev)
                g1 = work.tile([P, C], BF16, tag="g1", bufs=3, name=f"g1_{n}_{m}")
                nc.scalar.activation(g1, ps[:, 0:C], GELU_AF,
                                     bias=fc1b_sb[:, m:m + 1])
                g1_prev = g1
            fc2_mms(MLP // P - 1, g1_prev)
            for s in range(4):
                i = n * 4 + s
                tb = work.tile([P, C], F32, tag="tmp", bufs=3, name=f"res2_{i}")
                nc.vector.tensor_mul(tb, fps[s // 2][:, (s % 2) * 512:(s % 2) * 512 + 512], G2)
                nc.vector.tensor_add(tb, tb, GPB2)
                nc.vector.tensor_add(sx[i], sx[i], tb)
                nc.sync.dma_start(out_d[i], sx[i])

    nc.compile()
    return nc


def make_in_maps(inputs):
    bf = ml_dtypes.bfloat16
    f32 = np.float32
    x = np.asarray(inputs["x"], f32)
    c = np.asarray(inputs["c"], f32)
    qkv_w = np.asarray(inputs["qkv_w"], f32)
    qkv_b = np.asarray(inputs["qkv_b"], f32)
    proj_w = np.asarray(inputs["proj_w"], f32)
    proj_b = np.asarray(inputs["proj_b"], f32)
    ada_w = np.asarray(inputs["ada_w"], f32)
    ada_b = np.asarray(inputs["ada_b"], f32)
    fc1_w = np.asarray(inputs["fc1_w"], f32)
    fc1_b = np.asarray(inputs["fc1_b"], f32)
    fc2_w = np.asarray(inputs["fc2_w"], f32)
    fc2_b = np.asarray(inputs["fc2_b"], f32)
    ln = {k: np.asarray(inputs[k], f32) for k in
          ["ln1_w", "ln1_b", "ln2_w", "ln2_b"]}

    shared = {
        "ada_wt": np.ascontiguousarray(ada_w.T.reshape(KC, P, 6 * C)).astype(bf),
        "qkv_wt": np.ascontiguousarray(qkv_w.T.reshape(KC, P, 3 * C)).astype(bf),
        "proj_wt": np.ascontiguousarray(proj_w.T.reshape(KC, P, C)).astype(bf),
        "fc1_wt": np.ascontiguousarray(fc1_w.T.reshape(KC, P, MLP)).astype(bf),
        "fc2_wt": np.ascontiguousarray(fc2_w.T.reshape(MLP // P, P, C)).astype(bf),
        "qkv_b_qk": np.ascontiguousarray(qkv_b[:2 * C].reshape(8, P).T).astype(f32),
        "fc1_b_c": np.ascontiguousarray(fc1_b.reshape(MLP // P, P).T).astype(f32),
        "vb_row": qkv_b[2 * C:].reshape(1, C).astype(f32),
    }
    # host-folded constant rows (weights-only algebra; inputs never touched):
    #   W = ln_w*(1+mod_sc) where mod_sc = dev_sc + ada_b_sc
    #     = dev_sc*A + D with A = ln_w, D = ln_w*(1+ada_b_sc); similarly B, G.
    for br, (lnw, lnb, pb) in {1: (ln["ln1_w"], ln["ln1_b"], proj_b),
                               2: (ln["ln2_w"], ln["ln2_b"], fc2_b)}.items():
        o = (br - 1) * 3 * C
        sh_ab = ada_b[o:o + C]
        sc_ab = ada_b[o + C:o + 2 * C]
        g_ab = ada_b[o + 2 * C:o + 3 * C]
        shared[f"A{br}"] = lnw.reshape(1, C).astype(f32)
        shared[f"D{br}"] = (lnw * (1 + sc_ab)).reshape(1, C).astype(f32)
        shared[f"A2{br}"] = lnb.reshape(1, C).astype(f32)
        shared[f"E{br}"] = (lnb * (1 + sc_ab) + sh_ab).reshape(1, C).astype(f32)
        shared[f"pb{br}"] = pb.reshape(1, C).astype(f32)
        shared[f"gb{br}"] = g_ab.reshape(1, C).astype(f32)
    maps = []
    for b in range(B):
        m = dict(shared)
        m["x"] = np.ascontiguousarray(x[b].reshape(NT, P, C))
        m["c_col"] = np.ascontiguousarray(c[b].reshape(KC, P).T)
        maps.append(m)
    return maps


_CACHED_NC = None


def run(inputs, trace=False):
    global _CACHED_NC
    if _CACHED_NC is None:
        _CACHED_NC = build_program()
    maps = make_in_maps(inputs)
    res = run_bass_kernel_spmd(_CACHED_NC, maps, core_ids=list(range(B)),
                               trace=trace)
    out = np.stack([res.results[b]["out"].reshape(T, C) for b in range(B)])
    return out.astype(np.float32), res


def kernel(**inputs) -> np.ndarray:
    out, _ = run(inputs, trace=False)
    return out


# revision 21
# speedup vs baseline: 1.3821x; 1.1210x over previous
"""Trainium2 Bass kernel for the adaLN (DiT-style) dense transformer block.

Sharding: data-parallel over B — core b computes batch element b (B=8, 8 cores,
no collectives). Host-side prep is layout-only: weight transposes + bf16 casts.

Per-core dataflow (T=2048 tokens, C=512, H=8 heads, DH=64, MLP=2048):
  - LN stats + modulation in token-major (bn_stats over free dim, per-token
    scalars ride tensor_scalar per-partition operands)
  - big matmuls in feature-major (contraction dim on partitions); h is
    PE-transposed into feature-major after modulation
  - attention per head: S.T tiles [tk,tq] via lhsT=k.T, exp on ScalarE straight
    from PSUM (scale=1/8 folded in, no max-subtraction — logits are bounded),
    o via lhsT=[v|ones] so the softmax denominator rides the same matmul
  - proj/fc2 run "swapped" (lhsT=activations) so their outputs land
    token-major and the residual adds need no extra transpose
"""

import numpy as np
import ml_dtypes

import concourse.bass as bass
import concourse.bacc as bacc
import concourse.tile as tile
import concourse.mybir as mybir
from concourse.bass_utils import run_bass_kernel_spmd
from concourse.masks import make_identity

F32 = mybir.dt.float32
BF16 = mybir.dt.bfloat16
AF = mybir.ActivationFunctionType
ALU = mybir.AluOpType

B, T, C = 8, 2048, 512
H, DH, MLP = 8, 64, 4 * 512
P = 128
NT = T // P          # 16 token tiles
KC = C // P          # 4 feature chunks
NQ = T // 512        # 4 tq/tk column chunks of 512
EPS = 1e-5
GELU_AF = AF.Gelu_apprx_tanh  # test.py sim swaps to Tanh (CoreSim lacks gelu)


def build_program():
    nc = bacc.Bacc("TRN2", target_bir_lowering=False, debug=False)

    # ---- DRAM I/O ----
    x_d = nc.dram_tensor("x", [NT, P, C], F32, kind="ExternalInput").ap()
    c_col = nc.dram_tensor("c_col", [P, KC], F32, kind="ExternalInput").ap()
    ada_wt = nc.dram_tensor("ada_wt", [KC, P, 6 * C], BF16, kind="ExternalInput").ap()
    qkv_wt = nc.dram_tensor("qkv_wt", [KC, P, 3 * C], BF16, kind="ExternalInput").ap()
    proj_wt = nc.dram_tensor("proj_wt", [KC, P, C], BF16, kind="ExternalInput").ap()
    fc1_wt = nc.dram_tensor("fc1_wt", [KC, P, MLP], BF16, kind="ExternalInput").ap()
    fc2_wt = nc.dram_tensor("fc2_wt", [MLP // P, P, C], BF16, kind="ExternalInput").ap()
    qkv_b_qk = nc.dram_tensor("qkv_b_qk", [P, 8], F32, kind="ExternalInput").ap()
    fc1_b_c = nc.dram_tensor("fc1_b_c", [P, MLP // P], F32, kind="ExternalInput").ap()
    # host-folded constant rows (see make_in_maps): per branch br:
    #   A=ln_w, D=ln_w*(1+ada_b_sc), A2=ln_b, E=ln_b*(1+ada_b_sc)+ada_b_sh,
    #   pb=out-proj bias, gb=ada_b gate chunk; plus vb = qkv_b v-slice
    rows_d = {}
    for nm in (["vb_row"] +
               [f"{p}{br}" for br in (1, 2) for p in ("A", "D", "A2", "E", "pb", "gb")]):
        rows_d[nm] = nc.dram_tensor(nm, [1, C], F32, kind="ExternalInput").ap()
    out_d = nc.dram_tensor("out", [NT, P, C], F32, kind="ExternalOutput").ap()
    # DRAM bounce buffers: partition-broadcast DMA needs a DRAM source
    mod_scr = nc.dram_tensor("mod_scr", [6, C], F32).ap()
    rec_scr = nc.dram_tensor("rec_scr", [H * NQ, 512], F32).ap()

    from contextlib import ExitStack
    with tile.TileContext(nc) as tc, ExitStack() as ctx:
        consts = ctx.enter_context(tc.tile_pool(name="consts", bufs=1))
        wbig = ctx.enter_context(tc.tile_pool(name="wbig", bufs=8))
        wsmall = ctx.enter_context(tc.tile_pool(name="wsmall", bufs=16))
        bigT = ctx.enter_context(tc.tile_pool(name="bigT", bufs=8))
        qk_pool = ctx.enter_context(tc.tile_pool(name="qk", bufs=8))
        vpool = ctx.enter_context(tc.tile_pool(name="vp", bufs=NT))
        work = ctx.enter_context(tc.tile_pool(name="work", bufs=2))
        psum = ctx.enter_context(tc.tile_pool(name="ps", bufs=2, space="PSUM"))

        # ---- persistent SBUF loads (ada first: it gates the mod-vector chain) ----
        sc_col = consts.tile([P, KC], F32, name="sc_col")
        nc.sync.dma_start(sc_col, c_col)
        ada_sb = []
        for k in range(KC):
            halves = []
            for hh in range(2):
                w = wbig.tile([P, 3 * C], BF16, tag="wbig", name=f"ada{k}{hh}")
                nc.sync.dma_start(w, ada_wt[k][:, hh * 1536:(hh + 1) * 1536])
                halves.append(w)
            ada_sb.append(halves)
        sx = []
        for i in range(NT):
            t = consts.tile([P, C], F32, name=f"x{i}")
            nc.sync.dma_start(t, x_d[i])
            sx.append(t)
        ident = consts.tile([P, P], BF16, name="ident")
        make_identity(nc, ident)
        eps_t = consts.tile([P, 1], F32, name="eps_t")
        nc.gpsimd.memset(eps_t, EPS)
        qkvb_sb = consts.tile([P, 8], F32, name="qkvb_sb")
        nc.sync.dma_start(qkvb_sb, qkv_b_qk)
        fc1b_sb = consts.tile([P, MLP // P], F32, name="fc1b_sb")
        nc.sync.dma_start(fc1b_sb, fc1_b_c)

        # ---- phase 0: silu(c), mod = silu(c) @ ada_w.T + ada_b ----
        es_c = work.tile([P, KC], F32, tag="esc")
        nc.scalar.activation(es_c, sc_col, AF.Exp, scale=-1.0)
        nc.vector.tensor_scalar_add(es_c, es_c, 1.0)
        nc.vector.reciprocal(es_c, es_c)
        silu_f = work.tile([P, KC], F32, tag="siluf")
        nc.vector.tensor_mul(silu_f, sc_col, es_c)
        silu_b = consts.tile([P, KC], BF16, name="silu_b")
        nc.vector.tensor_copy(silu_b, silu_f)

        def bcast(dst, src_row):
            src = bass.AP(tensor=src_row.tensor, offset=src_row.offset,
                          ap=[[0, dst.shape[0]]] + list(src_row.ap[1:]))
            nc.sync.dma_start(out=dst, in_=src)

        def ada_mm_row(j):
            """mod chunk j (pre-ada_b) as a [1, C] PSUM row.
            chunks: 0=sh_msa 1=sc_msa 2=g_msa 3=sh_mlp 4=sc_mlp 5=g_mlp"""
            ps = psum.tile([P, 1024], F32, tag="sg", name=f"adaps{j}")
            for k in range(KC):
                hh, off = divmod(j * C, 1536)
                nc.tensor.matmul(ps[0:1, 0:C], silu_b[:, k:k + 1],
                                 ada_sb[k][hh][:, off:off + C],
                                 start=(k == 0), stop=(k == KC - 1))
            mrow = work.tile([1, C], F32, tag="mrow", bufs=2, name=f"mrow{j}")
            nc.vector.tensor_copy(mrow, ps[0:1, 0:C])
            nc.sync.dma_start(mod_scr[j:j + 1, :], mrow)
            return mod_scr[j:j + 1, :]

        def tmp_bc(src_row, nm):
            t = work.tile([P, C], F32, tag="tmp", bufs=3, name=nm)
            bcast(t, src_row)
            return t

        # modulation vectors, replicated [P, C] bf16:
        #   W = ln_w*(1+sc) = sc_dev*A + D     B = ln_b*(1+sc)+sh = sc_dev*A2 + sh_dev + E
        #   G = g_dev + gb                     GPB = G*pb
        # where *_dev are the device-computed silu(c)@ada_wT chunks.
        vecs = {}
        for br in (1, 2):
            base = (br - 1) * 3
            g_bc = tmp_bc(ada_mm_row(base + 2), f"gbc{br}")
            gb_bc = tmp_bc(rows_d[f"gb{br}"], f"gbbc{br}")
            G = consts.tile([P, C], BF16, name=f"G{br}")
            nc.vector.tensor_add(G, g_bc, gb_bc)
            pb_bc = tmp_bc(rows_d[f"pb{br}"], f"pbbc{br}")
            GPB = consts.tile([P, C], BF16, name=f"GPB{br}")
            nc.vector.tensor_mul(GPB, G, pb_bc)
            A_bc = tmp_bc(rows_d[f"A{br}"], f"abc{br}")
            D_bc = tmp_bc(rows_d[f"D{br}"], f"dbc{br}")
            sc_bc = tmp_bc(ada_mm_row(base + 1), f"scbc{br}")
            W = consts.tile([P, C], BF16, name=f"W{br}")
            nc.vector.tensor_mul(W, sc_bc, A_bc)
            nc.vector.tensor_add(W, W, D_bc)
            sh_bc = tmp_bc(ada_mm_row(base + 0), f"shbc{br}")
            A2_bc = tmp_bc(rows_d[f"A2{br}"], f"a2bc{br}")
            Bv = consts.tile([P, C], BF16, name=f"B{br}")
            nc.vector.tensor_mul(Bv, sc_bc, A2_bc)
            nc.vector.tensor_add(Bv, Bv, sh_bc)
            E_bc = tmp_bc(rows_d[f"E{br}"], f"ebc{br}")
            nc.vector.tensor_add(Bv, Bv, E_bc)
            vecs[br] = (W, Bv, G, GPB)
        (W1, B1, G1, GPB1), (W2, B2, G2, GPB2) = vecs[1], vecs[2]
        VB = consts.tile([P, C], BF16, name="VB")
        vb_bc = tmp_bc(rows_d["vb_row"], "vbbc")
        nc.vector.tensor_copy(VB, vb_bc)

        # remaining weights (wbig slots 9-16 evict ada after its matmuls)
        qkv_sb = []
        for k in range(KC):
            w = wbig.tile([P, 3 * C], BF16, tag="wbig", name=f"qkvw{k}")
            nc.sync.dma_start(w, qkv_wt[k])
            qkv_sb.append(w)
        fc1_sb = []
        for k in range(KC):
            w = wbig.tile([P, MLP], BF16, tag="wbig", name=f"fc1w{k}")
            nc.sync.dma_start(w, fc1_wt[k])
            fc1_sb.append(w)
        proj_sb = []
        for k in range(KC):
            w = wbig.tile([P, C], BF16, tag="wbig", name=f"projw{k}")
            nc.sync.dma_start(w, proj_wt[k])
            proj_sb.append(w)
        fc2_sb = []
        for k in range(MLP // P):
            w = wsmall.tile([P, C], BF16, tag="wsmall", name=f"fc2w{k}")
            nc.sync.dma_start(w, fc2_wt[k])
            fc2_sb.append(w)

        # ---- LN split into stats pass + apply pass ----
        def ln_stats(xt, i, stats_tag):
            st = work.tile([P, 6], F32, tag="st", bufs=2, name=f"st{stats_tag}{i}")
            nc.vector.bn_stats(st, xt)
            mv = work.tile([P, 2], F32, tag="mv", bufs=2, name=f"mv{stats_tag}{i}")
            nc.vector.bn_aggr(mv, st)
            rstd = work.tile([P, 1], F32, tag="rstd", bufs=NT,
                             name=f"rstd{stats_tag}{i}")
            nc.scalar.activation(rstd, mv[:, 1:2], AF.Ln, bias=eps_t)
            nc.scalar.activation(rstd, rstd, AF.Exp, scale=-0.5)
            negmr = work.tile([P, 1], F32, tag="negmr", bufs=NT,
                              name=f"negmr{stats_tag}{i}")
            nc.vector.tensor_scalar(negmr, mv[:, 0:1], rstd, -1.0,
                                    op0=ALU.mult, op1=ALU.mult)
            return rstd, negmr

        def ln_apply(xt, i, rstd, negmr, Wt, Bt, hT, stats_tag):
            t1 = work.tile([P, C], BF16, tag="t1", bufs=2, name=f"t1{stats_tag}{i}")
            nc.scalar.activation(t1, xt, AF.Identity, bias=negmr, scale=rstd)
            nc.vector.tensor_mul(t1, t1, Wt)
            hb = work.tile([P, C], BF16, tag="hb", bufs=2, name=f"hb{stats_tag}{i}")
            nc.vector.tensor_add(hb, t1, Bt)
            for j in range(KC):
                tp = psum.tile([P, P], BF16, tag="sg", name=f"tp{stats_tag}_{i}_{j}")
                nc.tensor.transpose(tp, hb[:, j * P:(j + 1) * P], ident)
                nc.any.tensor_copy(hT[j][:, i * P:(i + 1) * P], tp)

        h1T = [bigT.tile([P, T], BF16, tag="bigT", name=f"h1T{j}") for j in range(KC)]
        stats1 = [ln_stats(sx[i], i, "a") for i in range(NT)]
        for i in range(NT):
            ln_apply(sx[i], i, stats1[i][0], stats1[i][1], W1, B1, h1T, "a")

        # ---- qkv: q,k feature-major [8 x (P, T)]; v token-major interleaved ----
        qkT = [qk_pool.tile([P, T], BF16, tag="qk", name=f"qkT{m}") for m in range(8)]
        for m in range(8):
            prs = [psum.tile([P, 1024], F32, tag="oaccp", name=f"qkps{m}_{pp}")
                   for pp in range(2)]
            for k in range(KC):
                for n in range(NQ):
                    nc.tensor.matmul(prs[n // 2][:, (n % 2) * 512:(n % 2) * 512 + 512],
                                     qkv_sb[k][:, m * P:(m + 1) * P],
                                     h1T[k][:, n * 512:(n + 1) * 512],
                                     start=(k == 0), stop=(k == KC - 1))
            for pp in range(2):
                nc.scalar.activation(qkT[m][:, pp * 1024:(pp + 1) * 1024],
                                     prs[pp], AF.Identity,
                                     bias=qkvb_sb[:, m:m + 1])

        # v: out token-major [t, c_v], scattered into [128, 8, 65] (| ones)
        vtok = [vpool.tile([P, H * 65], BF16, tag="vtok", name=f"vtok{i}")
                for i in range(NT)]
        for i in range(NT):
            ps = psum.tile([P, 1024], F32, tag="sg", name=f"vps{i}")
            for k in range(KC):
                nc.tensor.matmul(ps[:, 0:C], h1T[k][:, i * P:(i + 1) * P],
                                 qkv_sb[k][:, 2 * C:3 * C],
                                 start=(k == 0), stop=(k == KC - 1))
            src = ps[:, 0:C].rearrange("p (h d) -> p h d", h=H)
            dst3 = vtok[i].rearrange("p (h d) -> p h d", d=65)[:, :, 0:DH]
            vb3 = VB.rearrange("p (h d) -> p h d", h=H)
            nc.vector.tensor_add(dst3, src, vb3)
            ones_col = vtok[i].rearrange("p (h d) -> p h d", d=65)[:, :, DH:65]
            nc.gpsimd.memset(ones_col, 1.0)

        # ---- attention ----
        oT = [bigT.tile([P, T], BF16, tag="bigT", name=f"oT{j}") for j in range(KC)]
        rc_pool = ctx.enter_context(tc.tile_pool(name="rc", bufs=2))
        for h in range(H):
            qh = qkT[h // 2][(h % 2) * DH:(h % 2) * DH + DH, :]
            kh = qkT[4 + h // 2][(h % 2) * DH:(h % 2) * DH + DH, :]
            for npair in range(2):
                oaccp = psum.tile([P, 1024], F32, tag="oaccp",
                                  name=f"oaccp{h}_{npair}")
                es_prev = None
                for tk in range(NT):
                    vsl = vtok[tk][:, h * 65:h * 65 + 65]
                    sg = psum.tile([P, 1024], F32, tag="sg", name=f"sg{h}_{npair}_{tk}")
                    for n2 in range(2):
                        n = 2 * npair + n2
                        nc.tensor.matmul(sg[:, n2 * 512:(n2 + 1) * 512],
                                         kh[:, tk * P:(tk + 1) * P],
                                         qh[:, n * 512:(n + 1) * 512],
                                         start=True, stop=True)
                    # o-matmuls run one tk behind so the in-order PE queue
                    # never waits on the exp of the current tk
                    if es_prev is not None:
                        vprev = vtok[tk - 1][:, h * 65:h * 65 + 65]
                        for n2 in range(2):
                            nc.tensor.matmul(
                                oaccp[0:65, n2 * 512:(n2 + 1) * 512], vprev,
                                es_prev[:, n2 * 512:(n2 + 1) * 512],
                                start=(tk - 1 == 0), stop=False)
                    es = work.tile([P, 1024], BF16, tag="es", bufs=3,
                                   name=f"es{h}_{npair}_{tk}")
                    nc.scalar.activation(es, sg, AF.Exp, scale=0.125)
                    es_prev = es
                vlast = vtok[NT - 1][:, h * 65:h * 65 + 65]
                for n2 in range(2):
                    nc.tensor.matmul(oaccp[0:65, n2 * 512:(n2 + 1) * 512], vlast,
                                     es_prev[:, n2 * 512:(n2 + 1) * 512],
                                     start=False, stop=True)
                o_un = rc_pool.tile([65, 1024], F32, tag="oun", bufs=1,
                                    name=f"oun{h}_{npair}")
                nc.vector.tensor_copy(o_un, oaccp[0:65, :])
                for n2 in range(2):
                    n = 2 * npair + n2
                    osl = o_un[:, n2 * 512:(n2 + 1) * 512]
                    rrow = rc_pool.tile([1, 512], F32, tag="rrow", bufs=2,
                                        name=f"rr{h}_{n}")
                    nc.vector.reciprocal(rrow, osl[DH:DH + 1, :])
                    ridx = h * NQ + n
                    nc.sync.dma_start(rec_scr[ridx:ridx + 1, :], rrow)
                    rbc = rc_pool.tile([DH, 512], F32, tag="rbc", bufs=1,
                                       name=f"rb{h}_{n}")
                    bcast(rbc, rec_scr[ridx:ridx + 1, :])
                    nc.vector.tensor_mul(
                        oT[h // 2][(h % 2) * DH:(h % 2) * DH + DH,
                                   n * 512:(n + 1) * 512],
                        osl[0:DH, :], rbc)

        # ---- proj (swapped: token-major out) + residual 1 (in-place x) ----
        for i in range(NT):
            ps = psum.tile([P, 1024], F32, tag="sg", name=f"prps{i}")
            for k in range(KC):
                nc.tensor.matmul(ps[:, 0:C], oT[k][:, i * P:(i + 1) * P],
                                 proj_sb[k], start=(k == 0), stop=(k == KC - 1))
            attn_sb = work.tile([P, C], BF16, tag="attnsb", bufs=2,
                                name=f"attnsb{i}")
            nc.scalar.copy(attn_sb, ps[:, 0:C])
            ta = work.tile([P, C], F32, tag="tmp", bufs=3, name=f"res1_{i}")
            nc.gpsimd.tensor_mul(ta, attn_sb, G1)
            nc.gpsimd.tensor_add(ta, ta, GPB1)
            nc.gpsimd.tensor_add(sx[i], sx[i], ta)

        # ---- LN2 + modulate + transpose (h2T reuses h1T slots) ----
        h2T = [bigT.tile([P, T], BF16, tag="bigT", name=f"h2T{j}") for j in range(KC)]
        for i in range(NT):
            r2, nm2 = ln_stats(sx[i], i, "b")
            ln_apply(sx[i], i, r2, nm2, W2, B2, h2T, "b")

        # ---- MLP per t-chunk; fc2 swapped -> token-major; residual 2 ----
        for n in range(NQ):
            fps = [psum.tile([P, 1024], F32, tag="oaccp", name=f"fps{n}_{sp}")
                   for sp in range(2)]

            def fc2_mms(m, g1t):
                for s in range(4):
                    nc.tensor.matmul(fps[s // 2][:, (s % 2) * 512:(s % 2) * 512 + 512],
                                     g1t[:, s * P:(s + 1) * P], fc2_sb[m],
                                     start=(m == 0), stop=(m == MLP // P - 1))

            g1_prev = None
            for m in range(MLP // P):
                ps = psum.tile([P, 1024], F32, tag="sg", name=f"f1ps{n}_{m}")
                for k in range(KC):
                    nc.tensor.matmul(ps[:, 0:C], fc1_sb[k][:, m * P:(m + 1) * P],
                                     h2T[k][:, n * 512:(n + 1) * 512],
                                     start=(k == 0), stop=(k == KC - 1))
                if g1_prev is not None:
                    fc2_mms(m - 1, g1_prev)
                g1 = work.tile([P, C], BF16, tag="g1", bufs=3, name=f"g1_{n}_{m}")
                nc.scalar.activation(g1, ps[:, 0:C], GELU_AF,
                                     bias=fc1b_sb[:, m:m + 1])
                g1_prev = g1
            fc2_mms(MLP // P - 1, g1_prev)
            for s in range(4):
                i = n * 4 + s
                mlp_sb = work.tile([P, C], BF16, tag="attnsb", bufs=2,
                                   name=f"mlpsb{i}")
                nc.scalar.copy(mlp_sb, fps[s // 2][:, (s % 2) * 512:(s % 2) * 512 + 512])
                tb = work.tile([P, C], F32, tag="tmp", bufs=3, name=f"res2_{i}")
                nc.gpsimd.tensor_mul(tb, mlp_sb, G2)
                nc.gpsimd.tensor_add(tb, tb, GPB2)
                nc.gpsimd.tensor_add(sx[i], sx[i], tb)
                nc.sync.dma_start(out_d[i], sx[i])

    nc.compile()
    return nc


def make_in_maps(inputs):
    bf = ml_dtypes.bfloat16
    f32 = np.float32
    x = np.asarray(inputs["x"], f32)
    c = np.asarray(inputs["c"], f32)
    qkv_w = np.asarray(inputs["qkv_w"], f32)
    qkv_b = np.asarray(inputs["qkv_b"], f32)
    proj_w = np.asarray(inputs["proj_w"], f32)
    proj_b = np.asarray(inputs["proj_b"], f32)
    ada_w = np.asarray(inputs["ada_w"], f32)
    ada_b = np.asarray(inputs["ada_b"], f32)
    fc1_w = np.asarray(inputs["fc1_w"], f32)
    fc1_b = np.asarray(inputs["fc1_b"], f32)
    fc2_w = np.asarray(inputs["fc2_w"], f32)
    fc2_b = np.asarray(inputs["fc2_b"], f32)
    ln = {k: np.asarray(inputs[k], f32) for k in
          ["ln1_w", "ln1_b", "ln2_w", "ln2_b"]}

    shared = {
        "ada_wt": np.ascontiguousarray(ada_w.T.reshape(KC, P, 6 * C)).astype(bf),
        "qkv_wt": np.ascontiguousarray(qkv_w.T.reshape(KC, P, 3 * C)).astype(bf),
        "proj_wt": np.ascontiguousarray(proj_w.T.reshape(KC, P, C)).astype(bf),
        "fc1_wt": np.ascontiguousarray(fc1_w.T.reshape(KC, P, MLP)).astype(bf),
        "fc2_wt": np.ascontiguousarray(fc2_w.T.reshape(MLP // P, P, C)).astype(bf),
        "qkv_b_qk": np.ascontiguousarray(qkv_b[:2 * C].reshape(8, P).T).astype(f32),
        "fc1_b_c": np.ascontiguousarray(fc1_b.reshape(MLP // P, P).T).astype(f32),
        "vb_row": qkv_b[2 * C:].reshape(1, C).astype(f32),
    }
    # host-folded constant rows (weights-only algebra; inputs never touched):
    #   W = ln_w*(1+mod_sc) where mod_sc = dev_sc + ada_b_sc
    #     = dev_sc*A + D with A = ln_w, D = ln_w*(1+ada_b_sc); similarly B, G.
    for br, (lnw, lnb, pb) in {1: (ln["ln1_w"], ln["ln1_b"], proj_b),
                               2: (ln["ln2_w"], ln["ln2_b"], fc2_b)}.items():
        o = (br - 1) * 3 * C
        sh_ab = ada_b[o:o + C]
        sc_ab = ada_b[o + C:o + 2 * C]
        g_ab = ada_b[o + 2 * C:o + 3 * C]
        shared[f"A{br}"] = lnw.reshape(1, C).astype(f32)
        shared[f"D{br}"] = (lnw * (1 + sc_ab)).reshape(1, C).astype(f32)
        shared[f"A2{br}"] = lnb.reshape(1, C).astype(f32)
        shared[f"E{br}"] = (lnb * (1 + sc_ab) + sh_ab).reshape(1, C).astype(f32)
        shared[f"pb{br}"] = pb.reshape(1, C).astype(f32)
        shared[f"gb{br}"] = g_ab.reshape(1, C).astype(f32)
    maps = []
    for b in range(B):
        m = dict(shared)
        m["x"] = np.ascontiguousarray(x[b].reshape(NT, P, C))
        m["c_col"] = np.ascontiguousarray(c[b].reshape(KC, P).T)
        maps.append(m)
    return maps


_CACHED_NC = None


def run(inputs, trace=False):
    global _CACHED_NC
    if _CACHED_NC is None:
        _CACHED_NC = build_program()
    maps = make_in_maps(inputs)
    res = run_bass_kernel_spmd(_CACHED_NC, maps, core_ids=list(range(B)),
                               trace=trace)
    out = np.stack([res.results[b]["out"].reshape(T, C) for b in range(B)])
    return out.astype(np.float32), res


def kernel(**inputs) -> np.ndarray:
    out, _ = run(inputs, trace=False)
    return out


# revision 25
# speedup vs baseline: 1.3834x; 1.0010x over previous
"""Trainium2 Bass kernel for the adaLN (DiT-style) dense transformer block.

Sharding: data-parallel over B — core b computes batch element b (B=8, 8 cores,
no collectives). Host-side prep is layout-only: weight transposes + bf16 casts.

Per-core dataflow (T=2048 tokens, C=512, H=8 heads, DH=64, MLP=2048):
  - LN stats + modulation in token-major (bn_stats over free dim, per-token
    scalars ride tensor_scalar per-partition operands)
  - big matmuls in feature-major (contraction dim on partitions); h is
    PE-transposed into feature-major after modulation
  - attention per head: S.T tiles [tk,tq] via lhsT=k.T, exp on ScalarE straight
    from PSUM (scale=1/8 folded in, no max-subtraction — logits are bounded),
    o via lhsT=[v|ones] so the softmax denominator rides the same matmul
  - proj/fc2 run "swapped" (lhsT=activations) so their outputs land
    token-major and the residual adds need no extra transpose
"""

import numpy as np
import ml_dtypes

import concourse.bass as bass
import concourse.bacc as bacc
import concourse.tile as tile
import concourse.mybir as mybir
from concourse.bass_utils import run_bass_kernel_spmd
from concourse.masks import make_identity

F32 = mybir.dt.float32
BF16 = mybir.dt.bfloat16
AF = mybir.ActivationFunctionType
ALU = mybir.AluOpType

B, T, C = 8, 2048, 512
H, DH, MLP = 8, 64, 4 * 512
P = 128
NT = T // P          # 16 token tiles
KC = C // P          # 4 feature chunks
NQ = T // 512        # 4 tq/tk column chunks of 512
EPS = 1e-5
GELU_AF = AF.Gelu_apprx_tanh  # test.py sim swaps to Tanh (CoreSim lacks gelu)


def build_program():
    nc = bacc.Bacc("TRN2", target_bir_lowering=False, debug=False)

    # ---- DRAM I/O ----
    x_d = nc.dram_tensor("x", [NT, P, C], F32, kind="ExternalInput").ap()
    c_col = nc.dram_tensor("c_col", [P, KC], F32, kind="ExternalInput").ap()
    ada_wt = nc.dram_tensor("ada_wt", [KC, P, 6 * C], BF16, kind="ExternalInput").ap()
    qkv_wt = nc.dram_tensor("qkv_wt", [KC, P, 3 * C], BF16, kind="ExternalInput").ap()
    proj_wt = nc.dram_tensor("proj_wt", [KC, P, C], BF16, kind="ExternalInput").ap()
    fc1_wt = nc.dram_tensor("fc1_wt", [KC, P, MLP], BF16, kind="ExternalInput").ap()
    fc2_wt = nc.dram_tensor("fc2_wt", [MLP // P, P, C], BF16, kind="ExternalInput").ap()
    qkv_b_qk = nc.dram_tensor("qkv_b_qk", [P, 8], F32, kind="ExternalInput").ap()
    fc1_b_c = nc.dram_tensor("fc1_b_c", [P, MLP // P], F32, kind="ExternalInput").ap()
    # host-folded constant rows (see make_in_maps): per branch br:
    #   A=ln_w, D=ln_w*(1+ada_b_sc), A2=ln_b, E=ln_b*(1+ada_b_sc)+ada_b_sh,
    #   pb=out-proj bias, gb=ada_b gate chunk; plus vb = qkv_b v-slice
    rows_d = {}
    for nm in (["vb_row"] +
               [f"{p}{br}" for br in (1, 2) for p in ("A", "D", "A2", "E", "pb", "gb")]):
        rows_d[nm] = nc.dram_tensor(nm, [1, C], F32, kind="ExternalInput").ap()
    out_d = nc.dram_tensor("out", [NT, P, C], F32, kind="ExternalOutput").ap()
    # DRAM bounce buffers: partition-broadcast DMA needs a DRAM source
    mod_scr = nc.dram_tensor("mod_scr", [6, C], F32).ap()
    rec_scr = nc.dram_tensor("rec_scr", [H * NQ, 512], F32).ap()

    from contextlib import ExitStack
    with tile.TileContext(nc) as tc, ExitStack() as ctx:
        consts = ctx.enter_context(tc.tile_pool(name="consts", bufs=1))
        wbig = ctx.enter_context(tc.tile_pool(name="wbig", bufs=8))
        wsmall = ctx.enter_context(tc.tile_pool(name="wsmall", bufs=16))
        bigT = ctx.enter_context(tc.tile_pool(name="bigT", bufs=8))
        qk_pool = ctx.enter_context(tc.tile_pool(name="qk", bufs=8))
        vpool = ctx.enter_context(tc.tile_pool(name="vp", bufs=NT))
        work = ctx.enter_context(tc.tile_pool(name="work", bufs=2))
        psum = ctx.enter_context(tc.tile_pool(name="ps", bufs=2, space="PSUM"))

        # ---- persistent SBUF loads (ada first: it gates the mod-vector chain) ----
        sc_col = consts.tile([P, KC], F32, name="sc_col")
        nc.sync.dma_start(sc_col, c_col)
        ada_sb = []
        for k in range(KC):
            halves = []
            for hh in range(2):
                w = wbig.tile([P, 3 * C], BF16, tag="wbig", name=f"ada{k}{hh}")
                nc.sync.dma_start(w, ada_wt[k][:, hh * 1536:(hh + 1) * 1536])
                halves.append(w)
            ada_sb.append(halves)
        sx = []
        for i in range(NT):
            t = consts.tile([P, C], F32, name=f"x{i}")
            nc.sync.dma_start(t, x_d[i])
            sx.append(t)
        ident = consts.tile([P, P], BF16, name="ident")
        make_identity(nc, ident)
        eps_t = consts.tile([P, 1], F32, name="eps_t")
        nc.gpsimd.memset(eps_t, EPS)
        qkvb_sb = consts.tile([P, 8], F32, name="qkvb_sb")
        nc.sync.dma_start(qkvb_sb, qkv_b_qk)
        fc1b_sb = consts.tile([P, MLP // P], F32, name="fc1b_sb")
        nc.sync.dma_start(fc1b_sb, fc1_b_c)

        # ---- phase 0: silu(c), mod = silu(c) @ ada_w.T + ada_b ----
        es_c = work.tile([P, KC], F32, tag="esc")
        nc.scalar.activation(es_c, sc_col, AF.Exp, scale=-1.0)
        nc.vector.tensor_scalar_add(es_c, es_c, 1.0)
        nc.vector.reciprocal(es_c, es_c)
        silu_f = work.tile([P, KC], F32, tag="siluf")
        nc.vector.tensor_mul(silu_f, sc_col, es_c)
        silu_b = consts.tile([P, KC], BF16, name="silu_b")
        nc.vector.tensor_copy(silu_b, silu_f)

        def bcast(dst, src_row):
            src = bass.AP(tensor=src_row.tensor, offset=src_row.offset,
                          ap=[[0, dst.shape[0]]] + list(src_row.ap[1:]))
            nc.sync.dma_start(out=dst, in_=src)

        def ada_mm_row(j):
            """mod chunk j (pre-ada_b) as a [1, C] PSUM row.
            chunks: 0=sh_msa 1=sc_msa 2=g_msa 3=sh_mlp 4=sc_mlp 5=g_mlp"""
            ps = psum.tile([P, 1024], F32, tag="sg", name=f"adaps{j}")
            for k in range(KC):
                hh, off = divmod(j * C, 1536)
                nc.tensor.matmul(ps[0:1, 0:C], silu_b[:, k:k + 1],
                                 ada_sb[k][hh][:, off:off + C],
                                 start=(k == 0), stop=(k == KC - 1))
            mrow = work.tile([1, C], F32, tag="mrow", bufs=2, name=f"mrow{j}")
            nc.vector.tensor_copy(mrow, ps[0:1, 0:C])
            nc.sync.dma_start(mod_scr[j:j + 1, :], mrow)
            return mod_scr[j:j + 1, :]

        def tmp_bc(src_row, nm):
            t = work.tile([P, C], F32, tag="tmp", bufs=3, name=nm)
            bcast(t, src_row)
            return t

        # modulation vectors, replicated [P, C] bf16:
        #   W = ln_w*(1+sc) = sc_dev*A + D     B = ln_b*(1+sc)+sh = sc_dev*A2 + sh_dev + E
        #   G = g_dev + gb                     GPB = G*pb
        # where *_dev are the device-computed silu(c)@ada_wT chunks.
        vecs = {}
        for br in (1, 2):
            base = (br - 1) * 3
            g_bc = tmp_bc(ada_mm_row(base + 2), f"gbc{br}")
            gb_bc = tmp_bc(rows_d[f"gb{br}"], f"gbbc{br}")
            G = consts.tile([P, C], BF16, name=f"G{br}")
            nc.vector.tensor_add(G, g_bc, gb_bc)
            pb_bc = tmp_bc(rows_d[f"pb{br}"], f"pbbc{br}")
            GPB = consts.tile([P, C], BF16, name=f"GPB{br}")
            nc.vector.tensor_mul(GPB, G, pb_bc)
            A_bc = tmp_bc(rows_d[f"A{br}"], f"abc{br}")
            D_bc = tmp_bc(rows_d[f"D{br}"], f"dbc{br}")
            sc_bc = tmp_bc(ada_mm_row(base + 1), f"scbc{br}")
            W = consts.tile([P, C], BF16, name=f"W{br}")
            nc.vector.tensor_mul(W, sc_bc, A_bc)
            nc.vector.tensor_add(W, W, D_bc)
            sh_bc = tmp_bc(ada_mm_row(base + 0), f"shbc{br}")
            A2_bc = tmp_bc(rows_d[f"A2{br}"], f"a2bc{br}")
            Bv = consts.tile([P, C], BF16, name=f"B{br}")
            nc.vector.tensor_mul(Bv, sc_bc, A2_bc)
            nc.vector.tensor_add(Bv, Bv, sh_bc)
            E_bc = tmp_bc(rows_d[f"E{br}"], f"ebc{br}")
            nc.vector.tensor_add(Bv, Bv, E_bc)
            vecs[br] = (W, Bv, G, GPB)
        (W1, B1, G1, GPB1), (W2, B2, G2, GPB2) = vecs[1], vecs[2]
        VB = consts.tile([P, C], BF16, name="VB")
        vb_bc = tmp_bc(rows_d["vb_row"], "vbbc")
        nc.vector.tensor_copy(VB, vb_bc)

        # remaining weights (wbig slots 9-16 evict ada after its matmuls)
        qkv_sb = []
        for k in range(KC):
            w = wbig.tile([P, 3 * C], BF16, tag="wbig", name=f"qkvw{k}")
            nc.sync.dma_start(w, qkv_wt[k])
            qkv_sb.append(w)
        fc1_sb = []
        for k in range(KC):
            w = wbig.tile([P, MLP], BF16, tag="wbig", name=f"fc1w{k}")
            nc.sync.dma_start(w, fc1_wt[k])
            fc1_sb.append(w)
        proj_sb = []
        for k in range(KC):
            w = wbig.tile([P, C], BF16, tag="wbig", name=f"projw{k}")
            nc.sync.dma_start(w, proj_wt[k])
            proj_sb.append(w)
        fc2_sb = []
        for k in range(MLP // P):
            w = wsmall.tile([P, C], BF16, tag="wsmall", name=f"fc2w{k}")
            nc.sync.dma_start(w, fc2_wt[k])
            fc2_sb.append(w)

        # ---- LN split into passes; Ln/Exp batched so ACT loads each
        # table set once per LN phase instead of per tile ----
        def ln_stats_all(tag):
            mvs, rstds, negmrs = [], [], []
            for i in range(NT):
                st = work.tile([P, 6], F32, tag="st", bufs=2, name=f"st{tag}{i}")
                nc.vector.bn_stats(st, sx[i])
                mv = work.tile([P, 2], F32, tag="mv", bufs=NT, name=f"mv{tag}{i}")
                nc.vector.bn_aggr(mv, st)
                mvs.append(mv)
            for i in range(NT):
                rstd = work.tile([P, 1], F32, tag="rstd", bufs=NT,
                                 name=f"rstd{tag}{i}")
                nc.scalar.activation(rstd, mvs[i][:, 1:2], AF.Ln, bias=eps_t)
                rstds.append(rstd)
            for i in range(NT):
                nc.scalar.activation(rstds[i], rstds[i], AF.Exp, scale=-0.5)
            for i in range(NT):
                negmr = work.tile([P, 1], F32, tag="negmr", bufs=NT,
                                  name=f"negmr{tag}{i}")
                nc.vector.tensor_scalar(negmr, mvs[i][:, 0:1], rstds[i], -1.0,
                                        op0=ALU.mult, op1=ALU.mult)
                negmrs.append(negmr)
            return rstds, negmrs

        def ln_apply(xt, i, rstd, negmr, Wt, Bt, hT, stats_tag):
            t1 = work.tile([P, C], BF16, tag="t1", bufs=2, name=f"t1{stats_tag}{i}")
            nc.scalar.activation(t1, xt, AF.Identity, bias=negmr, scale=rstd)
            nc.vector.tensor_mul(t1, t1, Wt)
            hb = work.tile([P, C], BF16, tag="hb", bufs=2, name=f"hb{stats_tag}{i}")
            nc.vector.tensor_add(hb, t1, Bt)
            for j in range(KC):
                tp = psum.tile([P, P], BF16, tag="sg", name=f"tp{stats_tag}_{i}_{j}")
                nc.tensor.transpose(tp, hb[:, j * P:(j + 1) * P], ident)
                nc.vector.tensor_copy(hT[j][:, i * P:(i + 1) * P], tp)

        h1T = [bigT.tile([P, T], BF16, tag="bigT", name=f"h1T{j}") for j in range(KC)]
        rstds1, negmrs1 = ln_stats_all("a")
        for i in range(NT):
            ln_apply(sx[i], i, rstds1[i], negmrs1[i], W1, B1, h1T, "a")

        # ---- qkv: q,k feature-major [8 x (P, T)]; v token-major interleaved ----
        # v: out token-major [t, c_v], scattered into [128, 8, 65] (| ones)
        vtok = [vpool.tile([P, H * 65], BF16, tag="vtok", name=f"vtok{i}")
                for i in range(NT)]
        for i in range(NT):
            ps = psum.tile([P, 1024], F32, tag="sg", name=f"vps{i}")
            for k in range(KC):
                nc.tensor.matmul(ps[:, 0:C], h1T[k][:, i * P:(i + 1) * P],
                                 qkv_sb[k][:, 2 * C:3 * C],
                                 start=(k == 0), stop=(k == KC - 1))
            src = ps[:, 0:C].rearrange("p (h d) -> p h d", h=H)
            dst3 = vtok[i].rearrange("p (h d) -> p h d", d=65)[:, :, 0:DH]
            vb3 = VB.rearrange("p (h d) -> p h d", h=H)
            nc.vector.tensor_add(dst3, src, vb3)
            ones_col = vtok[i].rearrange("p (h d) -> p h d", d=65)[:, :, DH:65]
            nc.gpsimd.memset(ones_col, 1.0)

        qkT = [qk_pool.tile([P, T], BF16, tag="qk", name=f"qkT{m}") for m in range(8)]
        for m in [0, 4, 1, 5, 2, 6, 3, 7]:
            prs = [psum.tile([P, 1024], F32, tag="oaccp", name=f"qkps{m}_{pp}")
                   for pp in range(2)]
            for k in range(KC):
                for n in range(NQ):
                    nc.tensor.matmul(prs[n // 2][:, (n % 2) * 512:(n % 2) * 512 + 512],
                                     qkv_sb[k][:, m * P:(m + 1) * P],
                                     h1T[k][:, n * 512:(n + 1) * 512],
                                     start=(k == 0), stop=(k == KC - 1))
            for pp in range(2):
                nc.scalar.activation(qkT[m][:, pp * 1024:(pp + 1) * 1024],
                                     prs[pp], AF.Identity,
                                     bias=qkvb_sb[:, m:m + 1])

        # ---- attention ----
        oT = [bigT.tile([P, T], BF16, tag="bigT", name=f"oT{j}") for j in range(KC)]
        rc_pool = ctx.enter_context(tc.tile_pool(name="rc", bufs=2))
        for h in range(H):
            qh = qkT[h // 2][(h % 2) * DH:(h % 2) * DH + DH, :]
            kh = qkT[4 + h // 2][(h % 2) * DH:(h % 2) * DH + DH, :]
            for npair in range(2):
                oaccp = psum.tile([P, 1024], F32, tag="oaccp",
                                  name=f"oaccp{h}_{npair}")
                es_prev = None
                for tk in range(NT):
                    vsl = vtok[tk][:, h * 65:h * 65 + 65]
                    sg = psum.tile([P, 1024], F32, tag="sg", name=f"sg{h}_{npair}_{tk}")
                    for n2 in range(2):
                        n = 2 * npair + n2
                        nc.tensor.matmul(sg[:, n2 * 512:(n2 + 1) * 512],
                                         kh[:, tk * P:(tk + 1) * P],
                                         qh[:, n * 512:(n + 1) * 512],
                                         start=True, stop=True)
                    # o-matmuls run one tk behind so the in-order PE queue
                    # never waits on the exp of the current tk
                    if es_prev is not None:
                        vprev = vtok[tk - 1][:, h * 65:h * 65 + 65]
                        for n2 in range(2):
                            nc.tensor.matmul(
                                oaccp[0:65, n2 * 512:(n2 + 1) * 512], vprev,
                                es_prev[:, n2 * 512:(n2 + 1) * 512],
                                start=(tk - 1 == 0), stop=False)
                    es = work.tile([P, 1024], BF16, tag="es", bufs=3,
                                   name=f"es{h}_{npair}_{tk}")
                    nc.scalar.activation(es, sg, AF.Exp, scale=0.125)
                    es_prev = es
                vlast = vtok[NT - 1][:, h * 65:h * 65 + 65]
                for n2 in range(2):
                    nc.tensor.matmul(oaccp[0:65, n2 * 512:(n2 + 1) * 512], vlast,
                                     es_prev[:, n2 * 512:(n2 + 1) * 512],
                                     start=False, stop=True)
                o_un = rc_pool.tile([65, 1024], F32, tag="oun", bufs=1,
                                    name=f"oun{h}_{npair}")
                nc.vector.tensor_copy(o_un, oaccp[0:65, :])
                for n2 in range(2):
                    n = 2 * npair + n2
                    osl = o_un[:, n2 * 512:(n2 + 1) * 512]
                    rrow = rc_pool.tile([1, 512], F32, tag="rrow", bufs=2,
                                        name=f"rr{h}_{n}")
                    nc.vector.reciprocal(rrow, osl[DH:DH + 1, :])
                    ridx = h * NQ + n
                    nc.sync.dma_start(rec_scr[ridx:ridx + 1, :], rrow)
                    rbc = rc_pool.tile([DH, 512], F32, tag="rbc", bufs=1,
                                       name=f"rb{h}_{n}")
                    bcast(rbc, rec_scr[ridx:ridx + 1, :])
                    nc.vector.tensor_mul(
                        oT[h // 2][(h % 2) * DH:(h % 2) * DH + DH,
                                   n * 512:(n + 1) * 512],
                        osl[0:DH, :], rbc)

        # ---- proj (swapped: token-major out) + residual 1 (in-place x) ----
        for i in range(NT):
            ps = psum.tile([P, 1024], F32, tag="sg", name=f"prps{i}")
            for k in range(KC):
                nc.tensor.matmul(ps[:, 0:C], oT[k][:, i * P:(i + 1) * P],
                                 proj_sb[k], start=(k == 0), stop=(k == KC - 1))
            attn_sb = work.tile([P, C], BF16, tag="attnsb", bufs=2,
                                name=f"attnsb{i}")
            nc.scalar.copy(attn_sb, ps[:, 0:C])
            ta = work.tile([P, C], F32, tag="tmp", bufs=3, name=f"res1_{i}")
            nc.gpsimd.tensor_mul(ta, attn_sb, G1)
            nc.gpsimd.tensor_add(ta, ta, GPB1)
            nc.gpsimd.tensor_add(sx[i], sx[i], ta)

        # ---- LN2 + modulate + transpose (h2T reuses h1T slots) ----
        h2T = [bigT.tile([P, T], BF16, tag="bigT", name=f"h2T{j}") for j in range(KC)]
        rstds2, negmrs2 = ln_stats_all("b")
        for i in range(NT):
            ln_apply(sx[i], i, rstds2[i], negmrs2[i], W2, B2, h2T, "b")

        # ---- MLP per t-chunk; fc2 swapped -> token-major; residual 2 ----
        for n in range(NQ):
            fps = [psum.tile([P, 1024], F32, tag="oaccp", name=f"fps{n}_{sp}")
                   for sp in range(2)]

            def fc2_mms(m, g1t):
                for s in range(4):
                    nc.tensor.matmul(fps[s // 2][:, (s % 2) * 512:(s % 2) * 512 + 512],
                                     g1t[:, s * P:(s + 1) * P], fc2_sb[m],
                                     start=(m == 0), stop=(m == MLP // P - 1))

            g1_prev = None
            for m in range(MLP // P):
                ps = psum.tile([P, 1024], F32, tag="sg", name=f"f1ps{n}_{m}")
                for k in range(KC):
                    nc.tensor.matmul(ps[:, 0:C], fc1_sb[k][:, m * P:(m + 1) * P],
                                     h2T[k][:, n * 512:(n + 1) * 512],
                                     start=(k == 0), stop=(k == KC - 1))
                if g1_prev is not None:
                    fc2_mms(m - 1, g1_prev)
                g1 = work.tile([P, C], BF16, tag="g1", bufs=3, name=f"g1_{n}_{m}")
                nc.scalar.activation(g1, ps[:, 0:C], GELU_AF,
                                     bias=fc1b_sb[:, m:m + 1])
                g1_prev = g1
            fc2_mms(MLP // P - 1, g1_prev)
            for s in range(4):
                i = n * 4 + s
                mlp_sb = work.tile([P, C], BF16, tag="attnsb", bufs=2,
                                   name=f"mlpsb{i}")
                nc.scalar.copy(mlp_sb, fps[s // 2][:, (s % 2) * 512:(s % 2) * 512 + 512])
                tb = work.tile([P, C], F32, tag="tmp", bufs=3, name=f"res2_{i}")
                nc.gpsimd.tensor_mul(tb, mlp_sb, G2)
                nc.gpsimd.tensor_add(tb, tb, GPB2)
                nc.gpsimd.tensor_add(sx[i], sx[i], tb)
                nc.sync.dma_start(out_d[i], sx[i])

    nc.compile()
    return nc


def make_in_maps(inputs):
    bf = ml_dtypes.bfloat16
    f32 = np.float32
    x = np.asarray(inputs["x"], f32)
    c = np.asarray(inputs["c"], f32)
    qkv_w = np.asarray(inputs["qkv_w"], f32)
    qkv_b = np.asarray(inputs["qkv_b"], f32)
    proj_w = np.asarray(inputs["proj_w"], f32)
    proj_b = np.asarray(inputs["proj_b"], f32)
    ada_w = np.asarray(inputs["ada_w"], f32)
    ada_b = np.asarray(inputs["ada_b"], f32)
    fc1_w = np.asarray(inputs["fc1_w"], f32)
    fc1_b = np.asarray(inputs["fc1_b"], f32)
    fc2_w = np.asarray(inputs["fc2_w"], f32)
    fc2_b = np.asarray(inputs["fc2_b"], f32)
    ln = {k: np.asarray(inputs[k], f32) for k in
          ["ln1_w", "ln1_b", "ln2_w", "ln2_b"]}

    shared = {
        "ada_wt": np.ascontiguousarray(ada_w.T.reshape(KC, P, 6 * C)).astype(bf),
        "qkv_wt": np.ascontiguousarray(qkv_w.T.reshape(KC, P, 3 * C)).astype(bf),
        "proj_wt": np.ascontiguousarray(proj_w.T.reshape(KC, P, C)).astype(bf),
        "fc1_wt": np.ascontiguousarray(fc1_w.T.reshape(KC, P, MLP)).astype(bf),
        "fc2_wt": np.ascontiguousarray(fc2_w.T.reshape(MLP // P, P, C)).astype(bf),
        "qkv_b_qk": np.ascontiguousarray(qkv_b[:2 * C].reshape(8, P).T).astype(f32),
        "fc1_b_c": np.ascontiguousarray(fc1_b.reshape(MLP // P, P).T).astype(f32),
        "vb_row": qkv_b[2 * C:].reshape(1, C).astype(f32),
    }
    # host-folded constant rows (weights-only algebra; inputs never touched):
    #   W = ln_w*(1+mod_sc) where mod_sc = dev_sc + ada_b_sc
    #     = dev_sc*A + D with A = ln_w, D = ln_w*(1+ada_b_sc); similarly B, G.
    for br, (lnw, lnb, pb) in {1: (ln["ln1_w"], ln["ln1_b"], proj_b),
                               2: (ln["ln2_w"], ln["ln2_b"], fc2_b)}.items():
        o = (br - 1) * 3 * C
        sh_ab = ada_b[o:o + C]
        sc_ab = ada_b[o + C:o + 2 * C]
        g_ab = ada_b[o + 2 * C:o + 3 * C]
        shared[f"A{br}"] = lnw.reshape(1, C).astype(f32)
        shared[f"D{br}"] = (lnw * (1 + sc_ab)).reshape(1, C).astype(f32)
        shared[f"A2{br}"] = lnb.reshape(1, C).astype(f32)
        shared[f"E{br}"] = (lnb * (1 + sc_ab) + sh_ab).reshape(1, C).astype(f32)
        shared[f"pb{br}"] = pb.reshape(1, C).astype(f32)
        shared[f"gb{br}"] = g_ab.reshape(1, C).astype(f32)
    maps = []
    for b in range(B):
        m = dict(shared)
        m["x"] = np.ascontiguousarray(x[b].reshape(NT, P, C))
        m["c_col"] = np.ascontiguousarray(c[b].reshape(KC, P).T)
        maps.append(m)
    return maps


_CACHED_NC = None


def run(inputs, trace=False):
    global _CACHED_NC
    if _CACHED_NC is None:
        _CACHED_NC = build_program()
    maps = make_in_maps(inputs)
    res = run_bass_kernel_spmd(_CACHED_NC, maps, core_ids=list(range(B)),
                               trace=trace)
    out = np.stack([res.results[b]["out"].reshape(T, C) for b in range(B)])
    return out.astype(np.float32), res


def kernel(**inputs) -> np.ndarray:
    out, _ = run(inputs, trace=False)
    return out


# revision 26
# speedup vs baseline: 1.4170x; 1.0243x over previous
"""Trainium2 Bass kernel for the adaLN (DiT-style) dense transformer block.

Sharding: data-parallel over B — core b computes batch element b (B=8, 8 cores,
no collectives). Host-side prep is layout-only: weight transposes + bf16 casts.

Per-core dataflow (T=2048 tokens, C=512, H=8 heads, DH=64, MLP=2048):
  - LN stats + modulation in token-major (bn_stats over free dim, per-token
    scalars ride tensor_scalar per-partition operands)
  - big matmuls in feature-major (contraction dim on partitions); h is
    PE-transposed into feature-major after modulation
  - attention per head: S.T tiles [tk,tq] via lhsT=k.T, exp on ScalarE straight
    from PSUM (scale=1/8 folded in, no max-subtraction — logits are bounded),
    o via lhsT=[v|ones] so the softmax denominator rides the same matmul
  - proj/fc2 run "swapped" (lhsT=activations) so their outputs land
    token-major and the residual adds need no extra transpose
"""

import numpy as np
import ml_dtypes

import concourse.bass as bass
import concourse.bacc as bacc
import concourse.tile as tile
import concourse.mybir as mybir
from concourse.bass_utils import run_bass_kernel_spmd
from concourse.masks import make_identity

F32 = mybir.dt.float32
BF16 = mybir.dt.bfloat16
AF = mybir.ActivationFunctionType
ALU = mybir.AluOpType

B, T, C = 8, 2048, 512
H, DH, MLP = 8, 64, 4 * 512
P = 128
NT = T // P          # 16 token tiles
KC = C // P          # 4 feature chunks
NQ = T // 512        # 4 tq/tk column chunks of 512
EPS = 1e-5
GELU_AF = AF.Gelu_apprx_tanh  # test.py sim swaps to Tanh (CoreSim lacks gelu)


def build_program():
    nc = bacc.Bacc("TRN2", target_bir_lowering=False, debug=False)

    # ---- DRAM I/O ----
    x_d = nc.dram_tensor("x", [NT, P, C], F32, kind="ExternalInput").ap()
    c_col = nc.dram_tensor("c_col", [P, KC], F32, kind="ExternalInput").ap()
    ada_wt = nc.dram_tensor("ada_wt", [KC, P, 6 * C], BF16, kind="ExternalInput").ap()
    qkv_wt = nc.dram_tensor("qkv_wt", [KC, P, 3 * C], BF16, kind="ExternalInput").ap()
    proj_wt = nc.dram_tensor("proj_wt", [KC, P, C], BF16, kind="ExternalInput").ap()
    fc1_wt = nc.dram_tensor("fc1_wt", [KC, P, MLP], BF16, kind="ExternalInput").ap()
    fc2_wt = nc.dram_tensor("fc2_wt", [MLP // P, P, C], BF16, kind="ExternalInput").ap()
    qkv_b_qk = nc.dram_tensor("qkv_b_qk", [P, 8], F32, kind="ExternalInput").ap()
    fc1_b_c = nc.dram_tensor("fc1_b_c", [P, MLP // P], F32, kind="ExternalInput").ap()
    # host-folded constant rows (see make_in_maps): per branch br:
    #   A=ln_w, D=ln_w*(1+ada_b_sc), A2=ln_b, E=ln_b*(1+ada_b_sc)+ada_b_sh,
    #   pb=out-proj bias, gb=ada_b gate chunk; plus vb = qkv_b v-slice
    rows_d = {}
    for nm in (["vb_row"] +
               [f"{p}{br}" for br in (1, 2) for p in ("A", "D", "A2", "E", "pb", "gb")]):
        rows_d[nm] = nc.dram_tensor(nm, [1, C], F32, kind="ExternalInput").ap()
    out_d = nc.dram_tensor("out", [NT, P, C], F32, kind="ExternalOutput").ap()
    # DRAM bounce buffers: partition-broadcast DMA needs a DRAM source
    mod_scr = nc.dram_tensor("mod_scr", [6, C], F32).ap()
    rec_scr = nc.dram_tensor("rec_scr", [H * NQ, 512], F32).ap()

    from contextlib import ExitStack
    with tile.TileContext(nc) as tc, ExitStack() as ctx:
        consts = ctx.enter_context(tc.tile_pool(name="consts", bufs=1))
        wbig = ctx.enter_context(tc.tile_pool(name="wbig", bufs=8))
        wsmall = ctx.enter_context(tc.tile_pool(name="wsmall", bufs=16))
        bigT = ctx.enter_context(tc.tile_pool(name="bigT", bufs=8))
        qk_pool = ctx.enter_context(tc.tile_pool(name="qk", bufs=8))
        vpool = ctx.enter_context(tc.tile_pool(name="vp", bufs=NT))
        work = ctx.enter_context(tc.tile_pool(name="work", bufs=2))
        psum = ctx.enter_context(tc.tile_pool(name="ps", bufs=2, space="PSUM"))

        # ---- persistent SBUF loads (ada first: it gates the mod-vector chain) ----
        sc_col = consts.tile([P, KC], F32, name="sc_col")
        nc.sync.dma_start(sc_col, c_col)
        ada_sb = []
        for k in range(KC):
            halves = []
            for hh in range(2):
                w = wbig.tile([P, 3 * C], BF16, tag="wbig", name=f"ada{k}{hh}")
                nc.sync.dma_start(w, ada_wt[k][:, hh * 1536:(hh + 1) * 1536])
                halves.append(w)
            ada_sb.append(halves)
        sx = []
        for i in range(NT):
            t = consts.tile([P, C], F32, name=f"x{i}")
            nc.scalar.dma_start(t, x_d[i])
            sx.append(t)
        ident = consts.tile([P, P], BF16, name="ident")
        make_identity(nc, ident)
        eps_t = consts.tile([P, 1], F32, name="eps_t")
        nc.gpsimd.memset(eps_t, EPS)
        qkvb_sb = consts.tile([P, 8], F32, name="qkvb_sb")
        nc.sync.dma_start(qkvb_sb, qkv_b_qk)
        fc1b_sb = consts.tile([P, MLP // P], F32, name="fc1b_sb")
        nc.sync.dma_start(fc1b_sb, fc1_b_c)

        # ---- phase 0: silu(c), mod = silu(c) @ ada_w.T + ada_b ----
        es_c = work.tile([P, KC], F32, tag="esc")
        nc.scalar.activation(es_c, sc_col, AF.Exp, scale=-1.0)
        nc.vector.tensor_scalar_add(es_c, es_c, 1.0)
        nc.vector.reciprocal(es_c, es_c)
        silu_f = work.tile([P, KC], F32, tag="siluf")
        nc.vector.tensor_mul(silu_f, sc_col, es_c)
        silu_b = consts.tile([P, KC], BF16, name="silu_b")
        nc.vector.tensor_copy(silu_b, silu_f)

        def bcast(dst, src_row):
            src = bass.AP(tensor=src_row.tensor, offset=src_row.offset,
                          ap=[[0, dst.shape[0]]] + list(src_row.ap[1:]))
            nc.sync.dma_start(out=dst, in_=src)

        def ada_mm_row(j):
            """mod chunk j (pre-ada_b) as a [1, C] PSUM row.
            chunks: 0=sh_msa 1=sc_msa 2=g_msa 3=sh_mlp 4=sc_mlp 5=g_mlp"""
            ps = psum.tile([P, 1024], F32, tag="sg", name=f"adaps{j}")
            for k in range(KC):
                hh, off = divmod(j * C, 1536)
                nc.tensor.matmul(ps[0:1, 0:C], silu_b[:, k:k + 1],
                                 ada_sb[k][hh][:, off:off + C],
                                 start=(k == 0), stop=(k == KC - 1))
            mrow = work.tile([1, C], F32, tag="mrow", bufs=2, name=f"mrow{j}")
            nc.vector.tensor_copy(mrow, ps[0:1, 0:C])
            nc.sync.dma_start(mod_scr[j:j + 1, :], mrow)
            return mod_scr[j:j + 1, :]

        def tmp_bc(src_row, nm):
            t = work.tile([P, C], F32, tag="tmp", bufs=3, name=nm)
            bcast(t, src_row)
            return t

        # modulation vectors, replicated [P, C] bf16:
        #   W = ln_w*(1+sc) = sc_dev*A + D     B = ln_b*(1+sc)+sh = sc_dev*A2 + sh_dev + E
        #   G = g_dev + gb                     GPB = G*pb
        # where *_dev are the device-computed silu(c)@ada_wT chunks.
        vecs = {}
        for br in (1, 2):
            base = (br - 1) * 3
            g_bc = tmp_bc(ada_mm_row(base + 2), f"gbc{br}")
            gb_bc = tmp_bc(rows_d[f"gb{br}"], f"gbbc{br}")
            G = consts.tile([P, C], BF16, name=f"G{br}")
            nc.vector.tensor_add(G, g_bc, gb_bc)
            pb_bc = tmp_bc(rows_d[f"pb{br}"], f"pbbc{br}")
            GPB = consts.tile([P, C], BF16, name=f"GPB{br}")
            nc.vector.tensor_mul(GPB, G, pb_bc)
            A_bc = tmp_bc(rows_d[f"A{br}"], f"abc{br}")
            D_bc = tmp_bc(rows_d[f"D{br}"], f"dbc{br}")
            sc_bc = tmp_bc(ada_mm_row(base + 1), f"scbc{br}")
            W = consts.tile([P, C], BF16, name=f"W{br}")
            nc.vector.tensor_mul(W, sc_bc, A_bc)
            nc.vector.tensor_add(W, W, D_bc)
            sh_bc = tmp_bc(ada_mm_row(base + 0), f"shbc{br}")
            A2_bc = tmp_bc(rows_d[f"A2{br}"], f"a2bc{br}")
            Bv = consts.tile([P, C], BF16, name=f"B{br}")
            nc.vector.tensor_mul(Bv, sc_bc, A2_bc)
            nc.vector.tensor_add(Bv, Bv, sh_bc)
            E_bc = tmp_bc(rows_d[f"E{br}"], f"ebc{br}")
            nc.vector.tensor_add(Bv, Bv, E_bc)
            vecs[br] = (W, Bv, G, GPB)
        (W1, B1, G1, GPB1), (W2, B2, G2, GPB2) = vecs[1], vecs[2]
        VB = consts.tile([P, C], BF16, name="VB")
        vb_bc = tmp_bc(rows_d["vb_row"], "vbbc")
        nc.vector.tensor_copy(VB, vb_bc)

        # remaining weights (wbig slots 9-16 evict ada after its matmuls)
        qkv_sb = []
        for k in range(KC):
            w = wbig.tile([P, 3 * C], BF16, tag="wbig", name=f"qkvw{k}")
            nc.scalar.dma_start(w, qkv_wt[k])
            qkv_sb.append(w)
        fc1_sb = []
        for k in range(KC):
            w = wbig.tile([P, MLP], BF16, tag="wbig", name=f"fc1w{k}")
            nc.scalar.dma_start(w, fc1_wt[k])
            fc1_sb.append(w)
        proj_sb = []
        for k in range(KC):
            w = wbig.tile([P, C], BF16, tag="wbig", name=f"projw{k}")
            nc.scalar.dma_start(w, proj_wt[k])
            proj_sb.append(w)
        fc2_sb = []
        for k in range(MLP // P):
            w = wsmall.tile([P, C], BF16, tag="wsmall", name=f"fc2w{k}")
            nc.scalar.dma_start(w, fc2_wt[k])
            fc2_sb.append(w)

        # ---- LN split into passes; Ln/Exp batched so ACT loads each
        # table set once per LN phase instead of per tile ----
        def ln_stats_all(tag):
            mvs, rstds, negmrs = [], [], []
            for i in range(NT):
                st = work.tile([P, 6], F32, tag="st", bufs=2, name=f"st{tag}{i}")
                nc.vector.bn_stats(st, sx[i])
                mv = work.tile([P, 2], F32, tag="mv", bufs=NT, name=f"mv{tag}{i}")
                nc.vector.bn_aggr(mv, st)
                mvs.append(mv)
            for i in range(NT):
                rstd = work.tile([P, 1], F32, tag="rstd", bufs=NT,
                                 name=f"rstd{tag}{i}")
                nc.scalar.activation(rstd, mvs[i][:, 1:2], AF.Ln, bias=eps_t)
                rstds.append(rstd)
            for i in range(NT):
                nc.scalar.activation(rstds[i], rstds[i], AF.Exp, scale=-0.5)
            for i in range(NT):
                negmr = work.tile([P, 1], F32, tag="negmr", bufs=NT,
                                  name=f"negmr{tag}{i}")
                nc.vector.tensor_scalar(negmr, mvs[i][:, 0:1], rstds[i], -1.0,
                                        op0=ALU.mult, op1=ALU.mult)
                negmrs.append(negmr)
            return rstds, negmrs

        def ln_apply(xt, i, rstd, negmr, Wt, Bt, hT, stats_tag):
            t1 = work.tile([P, C], BF16, tag="t1", bufs=2, name=f"t1{stats_tag}{i}")
            nc.scalar.activation(t1, xt, AF.Identity, bias=negmr, scale=rstd)
            nc.vector.tensor_mul(t1, t1, Wt)
            hb = work.tile([P, C], BF16, tag="hb", bufs=2, name=f"hb{stats_tag}{i}")
            nc.vector.tensor_add(hb, t1, Bt)
            for j in range(KC):
                tp = psum.tile([P, P], BF16, tag="sg", name=f"tp{stats_tag}_{i}_{j}")
                nc.tensor.transpose(tp, hb[:, j * P:(j + 1) * P], ident)
                nc.vector.tensor_copy(hT[j][:, i * P:(i + 1) * P], tp)

        h1T = [bigT.tile([P, T], BF16, tag="bigT", name=f"h1T{j}") for j in range(KC)]
        rstds1, negmrs1 = ln_stats_all("a")
        for i in range(NT):
            ln_apply(sx[i], i, rstds1[i], negmrs1[i], W1, B1, h1T, "a")

        # ---- qkv: q,k feature-major [8 x (P, T)]; v token-major interleaved ----
        # v: out token-major [t, c_v], scattered into [128, 8, 65] (| ones)
        vtok = [vpool.tile([P, H * 65], BF16, tag="vtok", name=f"vtok{i}")
                for i in range(NT)]
        for i in range(NT):
            ps = psum.tile([P, 1024], F32, tag="sg", name=f"vps{i}")
            for k in range(KC):
                nc.tensor.matmul(ps[:, 0:C], h1T[k][:, i * P:(i + 1) * P],
                                 qkv_sb[k][:, 2 * C:3 * C],
                                 start=(k == 0), stop=(k == KC - 1))
            src = ps[:, 0:C].rearrange("p (h d) -> p h d", h=H)
            dst3 = vtok[i].rearrange("p (h d) -> p h d", d=65)[:, :, 0:DH]
            vb3 = VB.rearrange("p (h d) -> p h d", h=H)
            nc.vector.tensor_add(dst3, src, vb3)
            ones_col = vtok[i].rearrange("p (h d) -> p h d", d=65)[:, :, DH:65]
            nc.gpsimd.memset(ones_col, 1.0)

        qkT = [qk_pool.tile([P, T], BF16, tag="qk", name=f"qkT{m}") for m in range(8)]
        for m in [0, 4, 1, 5, 2, 6, 3, 7]:
            prs = [psum.tile([P, 1024], F32, tag="oaccp", name=f"qkps{m}_{pp}")
                   for pp in range(2)]
            for k in range(KC):
                for n in range(NQ):
                    nc.tensor.matmul(prs[n // 2][:, (n % 2) * 512:(n % 2) * 512 + 512],
                                     qkv_sb[k][:, m * P:(m + 1) * P],
                                     h1T[k][:, n * 512:(n + 1) * 512],
                                     start=(k == 0), stop=(k == KC - 1))
            for pp in range(2):
                nc.scalar.activation(qkT[m][:, pp * 1024:(pp + 1) * 1024],
                                     prs[pp], AF.Identity,
                                     bias=qkvb_sb[:, m:m + 1])

        # ---- attention ----
        oT = [bigT.tile([P, T], BF16, tag="bigT", name=f"oT{j}") for j in range(KC)]
        rc_pool = ctx.enter_context(tc.tile_pool(name="rc", bufs=2))
        for h in range(H):
            qh = qkT[h // 2][(h % 2) * DH:(h % 2) * DH + DH, :]
            kh = qkT[4 + h // 2][(h % 2) * DH:(h % 2) * DH + DH, :]
            for npair in range(2):
                oaccp = psum.tile([P, 1024], F32, tag="oaccp",
                                  name=f"oaccp{h}_{npair}")
                es_prev = None
                for tk in range(NT):
                    vsl = vtok[tk][:, h * 65:h * 65 + 65]
                    sg = psum.tile([P, 1024], F32, tag="sg", name=f"sg{h}_{npair}_{tk}")
                    for n2 in range(2):
                        n = 2 * npair + n2
                        nc.tensor.matmul(sg[:, n2 * 512:(n2 + 1) * 512],
                                         kh[:, tk * P:(tk + 1) * P],
                                         qh[:, n * 512:(n + 1) * 512],
                                         start=True, stop=True)
                    # o-matmuls run one tk behind so the in-order PE queue
                    # never waits on the exp of the current tk
                    if es_prev is not None:
                        vprev = vtok[tk - 1][:, h * 65:h * 65 + 65]
                        for n2 in range(2):
                            nc.tensor.matmul(
                                oaccp[0:65, n2 * 512:(n2 + 1) * 512], vprev,
                                es_prev[:, n2 * 512:(n2 + 1) * 512],
                                start=(tk - 1 == 0), stop=False)
                    es = work.tile([P, 1024], BF16, tag="es", bufs=3,
                                   name=f"es{h}_{npair}_{tk}")
                    nc.scalar.activation(es, sg, AF.Exp, scale=0.125)
                    es_prev = es
                vlast = vtok[NT - 1][:, h * 65:h * 65 + 65]
                for n2 in range(2):
                    nc.tensor.matmul(oaccp[0:65, n2 * 512:(n2 + 1) * 512], vlast,
                                     es_prev[:, n2 * 512:(n2 + 1) * 512],
                                     start=False, stop=True)
                o_un = rc_pool.tile([65, 1024], F32, tag="oun", bufs=1,
                                    name=f"oun{h}_{npair}")
                nc.vector.tensor_copy(o_un, oaccp[0:65, :])
                for n2 in range(2):
                    n = 2 * npair + n2
                    osl = o_un[:, n2 * 512:(n2 + 1) * 512]
                    rrow = rc_pool.tile([1, 512], F32, tag="rrow", bufs=2,
                                        name=f"rr{h}_{n}")
                    nc.vector.reciprocal(rrow, osl[DH:DH + 1, :])
                    ridx = h * NQ + n
                    nc.sync.dma_start(rec_scr[ridx:ridx + 1, :], rrow)
                    rbc = rc_pool.tile([DH, 512], F32, tag="rbc", bufs=1,
                                       name=f"rb{h}_{n}")
                    bcast(rbc, rec_scr[ridx:ridx + 1, :])
                    nc.vector.tensor_mul(
                        oT[h // 2][(h % 2) * DH:(h % 2) * DH + DH,
                                   n * 512:(n + 1) * 512],
                        osl[0:DH, :], rbc)

        # GPB1 fold: x += G1*proj_b runs on GpSimd during attention
        for i in range(NT):
            nc.gpsimd.tensor_add(sx[i], sx[i], GPB1)

        # ---- proj (swapped: token-major out) + residual 1 (in-place x) ----
        for i in range(NT):
            ps = psum.tile([P, 1024], F32, tag="sg", name=f"prps{i}")
            for k in range(KC):
                nc.tensor.matmul(ps[:, 0:C], oT[k][:, i * P:(i + 1) * P],
                                 proj_sb[k], start=(k == 0), stop=(k == KC - 1))
            attn_sb = work.tile([P, C], BF16, tag="attnsb", bufs=2,
                                name=f"attnsb{i}")
            nc.scalar.copy(attn_sb, ps[:, 0:C])
            ta = work.tile([P, C], F32, tag="tmp", bufs=3, name=f"res1_{i}")
            nc.gpsimd.tensor_mul(ta, attn_sb, G1)
            nc.vector.tensor_add(sx[i], sx[i], ta)

        # ---- LN2 + modulate + transpose (h2T reuses h1T slots) ----
        h2T = [bigT.tile([P, T], BF16, tag="bigT", name=f"h2T{j}") for j in range(KC)]
        rstds2, negmrs2 = ln_stats_all("b")
        for i in range(NT):
            ln_apply(sx[i], i, rstds2[i], negmrs2[i], W2, B2, h2T, "b")
        # GPB2 fold after LN2 has consumed x2
        for i in range(NT):
            nc.gpsimd.tensor_add(sx[i], sx[i], GPB2)

        # ---- MLP per t-chunk; fc2 swapped -> token-major; residual 2 ----
        for n in range(NQ):
            fps = [psum.tile([P, 1024], F32, tag="oaccp", name=f"fps{n}_{sp}")
                   for sp in range(2)]

            def fc2_mms(m, g1t):
                for s in range(4):
                    nc.tensor.matmul(fps[s // 2][:, (s % 2) * 512:(s % 2) * 512 + 512],
                                     g1t[:, s * P:(s + 1) * P], fc2_sb[m],
                                     start=(m == 0), stop=(m == MLP // P - 1))

            g1_prev = None
            for m in range(MLP // P):
                ps = psum.tile([P, 1024], F32, tag="sg", name=f"f1ps{n}_{m}")
                for k in range(KC):
                    nc.tensor.matmul(ps[:, 0:C], fc1_sb[k][:, m * P:(m + 1) * P],
                                     h2T[k][:, n * 512:(n + 1) * 512],
                                     start=(k == 0), stop=(k == KC - 1))
                if g1_prev is not None:
                    fc2_mms(m - 1, g1_prev)
                g1 = work.tile([P, C], BF16, tag="g1", bufs=3, name=f"g1_{n}_{m}")
                nc.scalar.activation(g1, ps[:, 0:C], GELU_AF,
                                     bias=fc1b_sb[:, m:m + 1])
                g1_prev = g1
            fc2_mms(MLP // P - 1, g1_prev)
            for s in range(4):
                i = n * 4 + s
                mlp_sb = work.tile([P, C], BF16, tag="attnsb", bufs=2,
                                   name=f"mlpsb{i}")
                nc.scalar.copy(mlp_sb, fps[s // 2][:, (s % 2) * 512:(s % 2) * 512 + 512])
                tb = work.tile([P, C], F32, tag="tmp", bufs=3, name=f"res2_{i}")
                nc.gpsimd.tensor_mul(tb, mlp_sb, G2)
                nc.vector.tensor_add(sx[i], sx[i], tb)
                nc.sync.dma_start(out_d[i], sx[i])

    nc.compile()
    return nc


def make_in_maps(inputs):
    bf = ml_dtypes.bfloat16
    f32 = np.float32
    x = np.asarray(inputs["x"], f32)
    c = np.asarray(inputs["c"], f32)
    qkv_w = np.asarray(inputs["qkv_w"], f32)
    qkv_b = np.asarray(inputs["qkv_b"], f32)
    proj_w = np.asarray(inputs["proj_w"], f32)
    proj_b = np.asarray(inputs["proj_b"], f32)
    ada_w = np.asarray(inputs["ada_w"], f32)
    ada_b = np.asarray(inputs["ada_b"], f32)
    fc1_w = np.asarray(inputs["fc1_w"], f32)
    fc1_b = np.asarray(inputs["fc1_b"], f32)
    fc2_w = np.asarray(inputs["fc2_w"], f32)
    fc2_b = np.asarray(inputs["fc2_b"], f32)
    ln = {k: np.asarray(inputs[k], f32) for k in
          ["ln1_w", "ln1_b", "ln2_w", "ln2_b"]}

    shared = {
        "ada_wt": np.ascontiguousarray(ada_w.T.reshape(KC, P, 6 * C)).astype(bf),
        "qkv_wt": np.ascontiguousarray(qkv_w.T.reshape(KC, P, 3 * C)).astype(bf),
        "proj_wt": np.ascontiguousarray(proj_w.T.reshape(KC, P, C)).astype(bf),
        "fc1_wt": np.ascontiguousarray(fc1_w.T.reshape(KC, P, MLP)).astype(bf),
        "fc2_wt": np.ascontiguousarray(fc2_w.T.reshape(MLP // P, P, C)).astype(bf),
        "qkv_b_qk": np.ascontiguousarray(qkv_b[:2 * C].reshape(8, P).T).astype(f32),
        "fc1_b_c": np.ascontiguousarray(fc1_b.reshape(MLP // P, P).T).astype(f32),
        "vb_row": qkv_b[2 * C:].reshape(1, C).astype(f32),
    }
    # host-folded constant rows (weights-only algebra; inputs never touched):
    #   W = ln_w*(1+mod_sc) where mod_sc = dev_sc + ada_b_sc
    #     = dev_sc*A + D with A = ln_w, D = ln_w*(1+ada_b_sc); similarly B, G.
    for br, (lnw, lnb, pb) in {1: (ln["ln1_w"], ln["ln1_b"], proj_b),
                               2: (ln["ln2_w"], ln["ln2_b"], fc2_b)}.items():
        o = (br - 1) * 3 * C
        sh_ab = ada_b[o:o + C]
        sc_ab = ada_b[o + C:o + 2 * C]
        g_ab = ada_b[o + 2 * C:o + 3 * C]
        shared[f"A{br}"] = lnw.reshape(1, C).astype(f32)
        shared[f"D{br}"] = (lnw * (1 + sc_ab)).reshape(1, C).astype(f32)
        shared[f"A2{br}"] = lnb.reshape(1, C).astype(f32)
        shared[f"E{br}"] = (lnb * (1 + sc_ab) + sh_ab).reshape(1, C).astype(f32)
        shared[f"pb{br}"] = pb.reshape(1, C).astype(f32)
        shared[f"gb{br}"] = g_ab.reshape(1, C).astype(f32)
    maps = []
    for b in range(B):
        m = dict(shared)
        m["x"] = np.ascontiguousarray(x[b].reshape(NT, P, C))
        m["c_col"] = np.ascontiguousarray(c[b].reshape(KC, P).T)
        maps.append(m)
    return maps


_CACHED_NC = None


def run(inputs, trace=False):
    global _CACHED_NC
    if _CACHED_NC is None:
        _CACHED_NC = build_program()
    maps = make_in_maps(inputs)
    res = run_bass_kernel_spmd(_CACHED_NC, maps, core_ids=list(range(B)),
                               trace=trace)
    out = np.stack([res.results[b]["out"].reshape(T, C) for b in range(B)])
    return out.astype(np.float32), res


def kernel(**inputs) -> np.ndarray:
    out, _ = run(inputs, trace=False)
    return out


# revision 27
# speedup vs baseline: 1.4730x; 1.0395x over previous
"""Trainium2 Bass kernel for the adaLN (DiT-style) dense transformer block.

Sharding: data-parallel over B — core b computes batch element b (B=8, 8 cores,
no collectives). Host-side prep is layout-only: weight transposes + bf16 casts.

Per-core dataflow (T=2048 tokens, C=512, H=8 heads, DH=64, MLP=2048):
  - LN stats + modulation in token-major (bn_stats over free dim, per-token
    scalars ride tensor_scalar per-partition operands)
  - big matmuls in feature-major (contraction dim on partitions); h is
    PE-transposed into feature-major after modulation
  - attention per head: S.T tiles [tk,tq] via lhsT=k.T, exp on ScalarE straight
    from PSUM (scale=1/8 folded in, no max-subtraction — logits are bounded),
    o via lhsT=[v|ones] so the softmax denominator rides the same matmul
  - proj/fc2 run "swapped" (lhsT=activations) so their outputs land
    token-major and the residual adds need no extra transpose
"""

import numpy as np
import ml_dtypes

import concourse.bass as bass
import concourse.bacc as bacc
import concourse.hw_specs as _hw_specs

# Route Exp and Ln to the one table set that holds BOTH
# (natural_log_exp_and_others). The default first-match assignment puts Exp in
# exp_and_others and Ln in natural_log, so every rstd = exp(-ln(v)/2) pair
# costs two 1.3us ACT table reloads. Blank those two sets (positions kept so
# act_func_set_ids stay aligned with act_info.json) and both functions
# first-match the combined set -> zero reloads.
_orig_get_tables = _hw_specs.get_activation_tables

def _patched_get_tables(arch):
    t = _orig_get_tables(arch)
    for nm in ("exp_and_others", "natural_log"):
        if nm in t:
            t[nm] = set()
    return t

_hw_specs.get_activation_tables = _patched_get_tables
bacc.get_activation_tables = _patched_get_tables
import concourse.tile as tile
import concourse.mybir as mybir
from concourse.bass_utils import run_bass_kernel_spmd
from concourse.masks import make_identity

F32 = mybir.dt.float32
BF16 = mybir.dt.bfloat16
AF = mybir.ActivationFunctionType
ALU = mybir.AluOpType

B, T, C = 8, 2048, 512
H, DH, MLP = 8, 64, 4 * 512
P = 128
NT = T // P          # 16 token tiles
KC = C // P          # 4 feature chunks
NQ = T // 512        # 4 tq/tk column chunks of 512
EPS = 1e-5
GELU_AF = AF.Gelu_apprx_tanh  # test.py sim swaps to Tanh (CoreSim lacks gelu)


def build_program():
    nc = bacc.Bacc("TRN2", target_bir_lowering=False, debug=False)

    # ---- DRAM I/O ----
    x_d = nc.dram_tensor("x", [NT, P, C], F32, kind="ExternalInput").ap()
    c_col = nc.dram_tensor("c_col", [P, KC], F32, kind="ExternalInput").ap()
    ada_wt = nc.dram_tensor("ada_wt", [KC, P, 6 * C], BF16, kind="ExternalInput").ap()
    qkv_wt = nc.dram_tensor("qkv_wt", [KC, P, 3 * C], BF16, kind="ExternalInput").ap()
    proj_wt = nc.dram_tensor("proj_wt", [KC, P, C], BF16, kind="ExternalInput").ap()
    fc1_wt = nc.dram_tensor("fc1_wt", [KC, P, MLP], BF16, kind="ExternalInput").ap()
    fc2_wt = nc.dram_tensor("fc2_wt", [MLP // P, P, C], BF16, kind="ExternalInput").ap()
    qkv_b_qk = nc.dram_tensor("qkv_b_qk", [P, 8], F32, kind="ExternalInput").ap()
    fc1_b_c = nc.dram_tensor("fc1_b_c", [P, MLP // P], F32, kind="ExternalInput").ap()
    # host-folded constant rows (see make_in_maps): per branch br:
    #   A=ln_w, D=ln_w*(1+ada_b_sc), A2=ln_b, E=ln_b*(1+ada_b_sc)+ada_b_sh,
    #   pb=out-proj bias, gb=ada_b gate chunk; plus vb = qkv_b v-slice
    rows_d = {}
    for nm in (["vb_row"] +
               [f"{p}{br}" for br in (1, 2) for p in ("A", "D", "A2", "E", "pb", "gb")]):
        rows_d[nm] = nc.dram_tensor(nm, [1, C], F32, kind="ExternalInput").ap()
    out_d = nc.dram_tensor("out", [NT, P, C], F32, kind="ExternalOutput").ap()
    # DRAM bounce buffers: partition-broadcast DMA needs a DRAM source
    mod_scr = nc.dram_tensor("mod_scr", [6, C], F32).ap()
    rec_scr = nc.dram_tensor("rec_scr", [H * NQ, 512], F32).ap()

    from contextlib import ExitStack
    with tile.TileContext(nc) as tc, ExitStack() as ctx:
        consts = ctx.enter_context(tc.tile_pool(name="consts", bufs=1))
        wbig = ctx.enter_context(tc.tile_pool(name="wbig", bufs=8))
        wsmall = ctx.enter_context(tc.tile_pool(name="wsmall", bufs=16))
        bigT = ctx.enter_context(tc.tile_pool(name="bigT", bufs=8))
        qk_pool = ctx.enter_context(tc.tile_pool(name="qk", bufs=8))
        vpool = ctx.enter_context(tc.tile_pool(name="vp", bufs=NT))
        work = ctx.enter_context(tc.tile_pool(name="work", bufs=2))
        psum = ctx.enter_context(tc.tile_pool(name="ps", bufs=2, space="PSUM"))

        # ---- persistent SBUF loads (ada first: it gates the mod-vector chain) ----
        sc_col = consts.tile([P, KC], F32, name="sc_col")
        nc.sync.dma_start(sc_col, c_col)
        ada_sb = []
        for k in range(KC):
            halves = []
            for hh in range(2):
                w = wbig.tile([P, 3 * C], BF16, tag="wbig", name=f"ada{k}{hh}")
                nc.sync.dma_start(w, ada_wt[k][:, hh * 1536:(hh + 1) * 1536])
                halves.append(w)
            ada_sb.append(halves)
        sx = []
        for i in range(NT):
            t = consts.tile([P, C], F32, name=f"x{i}")
            nc.scalar.dma_start(t, x_d[i])
            sx.append(t)
        ident = consts.tile([P, P], BF16, name="ident")
        make_identity(nc, ident)
        eps_t = consts.tile([P, 1], F32, name="eps_t")
        nc.gpsimd.memset(eps_t, EPS)
        qkvb_sb = consts.tile([P, 8], F32, name="qkvb_sb")
        nc.sync.dma_start(qkvb_sb, qkv_b_qk)
        fc1b_sb = consts.tile([P, MLP // P], F32, name="fc1b_sb")
        nc.sync.dma_start(fc1b_sb, fc1_b_c)

        # ---- phase 0: silu(c), mod = silu(c) @ ada_w.T + ada_b ----
        es_c = work.tile([P, KC], F32, tag="esc")
        nc.scalar.activation(es_c, sc_col, AF.Exp, scale=-1.0)
        nc.vector.tensor_scalar_add(es_c, es_c, 1.0)
        nc.vector.reciprocal(es_c, es_c)
        silu_f = work.tile([P, KC], F32, tag="siluf")
        nc.vector.tensor_mul(silu_f, sc_col, es_c)
        silu_b = consts.tile([P, KC], BF16, name="silu_b")
        nc.vector.tensor_copy(silu_b, silu_f)

        def bcast(dst, src_row):
            src = bass.AP(tensor=src_row.tensor, offset=src_row.offset,
                          ap=[[0, dst.shape[0]]] + list(src_row.ap[1:]))
            nc.sync.dma_start(out=dst, in_=src)

        def ada_mm_row(j):
            """mod chunk j (pre-ada_b) as a [1, C] PSUM row.
            chunks: 0=sh_msa 1=sc_msa 2=g_msa 3=sh_mlp 4=sc_mlp 5=g_mlp"""
            ps = psum.tile([P, 1024], F32, tag="sg", name=f"adaps{j}")
            for k in range(KC):
                hh, off = divmod(j * C, 1536)
                nc.tensor.matmul(ps[0:1, 0:C], silu_b[:, k:k + 1],
                                 ada_sb[k][hh][:, off:off + C],
                                 start=(k == 0), stop=(k == KC - 1))
            mrow = work.tile([1, C], F32, tag="mrow", bufs=2, name=f"mrow{j}")
            nc.vector.tensor_copy(mrow, ps[0:1, 0:C])
            nc.sync.dma_start(mod_scr[j:j + 1, :], mrow)
            return mod_scr[j:j + 1, :]

        def tmp_bc(src_row, nm):
            t = work.tile([P, C], F32, tag="tmp", bufs=3, name=nm)
            bcast(t, src_row)
            return t

        # modulation vectors, replicated [P, C] bf16:
        #   W = ln_w*(1+sc) = sc_dev*A + D     B = ln_b*(1+sc)+sh = sc_dev*A2 + sh_dev + E
        #   G = g_dev + gb                     GPB = G*pb
        # where *_dev are the device-computed silu(c)@ada_wT chunks.
        vecs = {}
        for br in (1, 2):
            base = (br - 1) * 3
            g_bc = tmp_bc(ada_mm_row(base + 2), f"gbc{br}")
            gb_bc = tmp_bc(rows_d[f"gb{br}"], f"gbbc{br}")
            G = consts.tile([P, C], BF16, name=f"G{br}")
            nc.vector.tensor_add(G, g_bc, gb_bc)
            pb_bc = tmp_bc(rows_d[f"pb{br}"], f"pbbc{br}")
            GPB = consts.tile([P, C], BF16, name=f"GPB{br}")
            nc.vector.tensor_mul(GPB, G, pb_bc)
            A_bc = tmp_bc(rows_d[f"A{br}"], f"abc{br}")
            D_bc = tmp_bc(rows_d[f"D{br}"], f"dbc{br}")
            sc_bc = tmp_bc(ada_mm_row(base + 1), f"scbc{br}")
            W = consts.tile([P, C], BF16, name=f"W{br}")
            nc.vector.tensor_mul(W, sc_bc, A_bc)
            nc.vector.tensor_add(W, W, D_bc)
            sh_bc = tmp_bc(ada_mm_row(base + 0), f"shbc{br}")
            A2_bc = tmp_bc(rows_d[f"A2{br}"], f"a2bc{br}")
            Bv = consts.tile([P, C], BF16, name=f"B{br}")
            nc.vector.tensor_mul(Bv, sc_bc, A2_bc)
            nc.vector.tensor_add(Bv, Bv, sh_bc)
            E_bc = tmp_bc(rows_d[f"E{br}"], f"ebc{br}")
            nc.vector.tensor_add(Bv, Bv, E_bc)
            vecs[br] = (W, Bv, G, GPB)
        (W1, B1, G1, GPB1), (W2, B2, G2, GPB2) = vecs[1], vecs[2]
        VB = consts.tile([P, C], BF16, name="VB")
        vb_bc = tmp_bc(rows_d["vb_row"], "vbbc")
        nc.vector.tensor_copy(VB, vb_bc)

        # remaining weights (wbig slots 9-16 evict ada after its matmuls)
        qkv_sb = []
        for k in range(KC):
            w = wbig.tile([P, 3 * C], BF16, tag="wbig", name=f"qkvw{k}")
            nc.scalar.dma_start(w, qkv_wt[k])
            qkv_sb.append(w)
        fc1_sb = []
        for k in range(KC):
            w = wbig.tile([P, MLP], BF16, tag="wbig", name=f"fc1w{k}")
            nc.scalar.dma_start(w, fc1_wt[k])
            fc1_sb.append(w)
        proj_sb = []
        for k in range(KC):
            w = wbig.tile([P, C], BF16, tag="wbig", name=f"projw{k}")
            nc.scalar.dma_start(w, proj_wt[k])
            proj_sb.append(w)
        fc2_sb = []
        for k in range(MLP // P):
            w = wsmall.tile([P, C], BF16, tag="wsmall", name=f"fc2w{k}")
            nc.scalar.dma_start(w, fc2_wt[k])
            fc2_sb.append(w)

        # ---- LN split into passes; Ln/Exp batched so ACT loads each
        # table set once per LN phase instead of per tile ----
        def ln_stats_all(tag):
            mvs, rstds, negmrs = [], [], []
            for i in range(NT):
                st = work.tile([P, 6], F32, tag="st", bufs=2, name=f"st{tag}{i}")
                nc.vector.bn_stats(st, sx[i])
                mv = work.tile([P, 2], F32, tag="mv", bufs=NT, name=f"mv{tag}{i}")
                nc.vector.bn_aggr(mv, st)
                mvs.append(mv)
            for i in range(NT):
                rstd = work.tile([P, 1], F32, tag="rstd", bufs=NT,
                                 name=f"rstd{tag}{i}")
                nc.scalar.activation(rstd, mvs[i][:, 1:2], AF.Ln, bias=eps_t)
                rstds.append(rstd)
            for i in range(NT):
                nc.scalar.activation(rstds[i], rstds[i], AF.Exp, scale=-0.5)
            for i in range(NT):
                negmr = work.tile([P, 1], F32, tag="negmr", bufs=NT,
                                  name=f"negmr{tag}{i}")
                nc.vector.tensor_scalar(negmr, mvs[i][:, 0:1], rstds[i], -1.0,
                                        op0=ALU.mult, op1=ALU.mult)
                negmrs.append(negmr)
            return rstds, negmrs

        def ln_apply(xt, i, rstd, negmr, Wt, Bt, hT, stats_tag):
            t1 = work.tile([P, C], BF16, tag="t1", bufs=2, name=f"t1{stats_tag}{i}")
            nc.scalar.activation(t1, xt, AF.Identity, bias=negmr, scale=rstd)
            nc.vector.tensor_mul(t1, t1, Wt)
            hb = work.tile([P, C], BF16, tag="hb", bufs=2, name=f"hb{stats_tag}{i}")
            nc.vector.tensor_add(hb, t1, Bt)
            for j in range(KC):
                tp = psum.tile([P, P], BF16, tag="sg", name=f"tp{stats_tag}_{i}_{j}")
                nc.tensor.transpose(tp, hb[:, j * P:(j + 1) * P], ident)
                nc.vector.tensor_copy(hT[j][:, i * P:(i + 1) * P], tp)

        h1T = [bigT.tile([P, T], BF16, tag="bigT", name=f"h1T{j}") for j in range(KC)]
        rstds1, negmrs1 = ln_stats_all("a")
        for i in range(NT):
            ln_apply(sx[i], i, rstds1[i], negmrs1[i], W1, B1, h1T, "a")

        # ---- qkv: q,k feature-major [8 x (P, T)]; v token-major interleaved ----
        # v: out token-major [t, c_v], scattered into [128, 8, 65] (| ones)
        vtok = [vpool.tile([P, H * 65], BF16, tag="vtok", name=f"vtok{i}")
                for i in range(NT)]
        for i in range(NT):
            ps = psum.tile([P, 1024], F32, tag="sg", name=f"vps{i}")
            for k in range(KC):
                nc.tensor.matmul(ps[:, 0:C], h1T[k][:, i * P:(i + 1) * P],
                                 qkv_sb[k][:, 2 * C:3 * C],
                                 start=(k == 0), stop=(k == KC - 1))
            src = ps[:, 0:C].rearrange("p (h d) -> p h d", h=H)
            dst3 = vtok[i].rearrange("p (h d) -> p h d", d=65)[:, :, 0:DH]
            vb3 = VB.rearrange("p (h d) -> p h d", h=H)
            nc.vector.tensor_add(dst3, src, vb3)
            ones_col = vtok[i].rearrange("p (h d) -> p h d", d=65)[:, :, DH:65]
            nc.gpsimd.memset(ones_col, 1.0)

        qkT = [qk_pool.tile([P, T], BF16, tag="qk", name=f"qkT{m}") for m in range(8)]
        for m in [0, 4, 1, 5, 2, 6, 3, 7]:
            prs = [psum.tile([P, 1024], F32, tag="oaccp", name=f"qkps{m}_{pp}")
                   for pp in range(2)]
            for k in range(KC):
                for n in range(NQ):
                    nc.tensor.matmul(prs[n // 2][:, (n % 2) * 512:(n % 2) * 512 + 512],
                                     qkv_sb[k][:, m * P:(m + 1) * P],
                                     h1T[k][:, n * 512:(n + 1) * 512],
                                     start=(k == 0), stop=(k == KC - 1))
            for pp in range(2):
                nc.scalar.activation(qkT[m][:, pp * 1024:(pp + 1) * 1024],
                                     prs[pp], AF.Identity,
                                     bias=qkvb_sb[:, m:m + 1])

        # ---- attention ----
        oT = [bigT.tile([P, T], BF16, tag="bigT", name=f"oT{j}") for j in range(KC)]
        rc_pool = ctx.enter_context(tc.tile_pool(name="rc", bufs=2))
        for h in range(H):
            qh = qkT[h // 2][(h % 2) * DH:(h % 2) * DH + DH, :]
            kh = qkT[4 + h // 2][(h % 2) * DH:(h % 2) * DH + DH, :]
            for npair in range(2):
                oaccp = psum.tile([P, 1024], F32, tag="oaccp",
                                  name=f"oaccp{h}_{npair}")
                es_prev = None
                for tk in range(NT):
                    vsl = vtok[tk][:, h * 65:h * 65 + 65]
                    sg = psum.tile([P, 1024], F32, tag="sg", name=f"sg{h}_{npair}_{tk}")
                    for n2 in range(2):
                        n = 2 * npair + n2
                        nc.tensor.matmul(sg[:, n2 * 512:(n2 + 1) * 512],
                                         kh[:, tk * P:(tk + 1) * P],
                                         qh[:, n * 512:(n + 1) * 512],
                                         start=True, stop=True)
                    # o-matmuls run one tk behind so the in-order PE queue
                    # never waits on the exp of the current tk
                    if es_prev is not None:
                        vprev = vtok[tk - 1][:, h * 65:h * 65 + 65]
                        for n2 in range(2):
                            nc.tensor.matmul(
                                oaccp[0:65, n2 * 512:(n2 + 1) * 512], vprev,
                                es_prev[:, n2 * 512:(n2 + 1) * 512],
                                start=(tk - 1 == 0), stop=False)
                    es = work.tile([P, 1024], BF16, tag="es", bufs=3,
                                   name=f"es{h}_{npair}_{tk}")
                    nc.scalar.activation(es, sg, AF.Exp, scale=0.125)
                    es_prev = es
                vlast = vtok[NT - 1][:, h * 65:h * 65 + 65]
                for n2 in range(2):
                    nc.tensor.matmul(oaccp[0:65, n2 * 512:(n2 + 1) * 512], vlast,
                                     es_prev[:, n2 * 512:(n2 + 1) * 512],
                                     start=False, stop=True)
                o_un = rc_pool.tile([65, 1024], F32, tag="oun", bufs=1,
                                    name=f"oun{h}_{npair}")
                nc.vector.tensor_copy(o_un, oaccp[0:65, :])
                for n2 in range(2):
                    n = 2 * npair + n2
                    osl = o_un[:, n2 * 512:(n2 + 1) * 512]
                    rrow = rc_pool.tile([1, 512], F32, tag="rrow", bufs=2,
                                        name=f"rr{h}_{n}")
                    nc.vector.reciprocal(rrow, osl[DH:DH + 1, :])
                    ridx = h * NQ + n
                    nc.sync.dma_start(rec_scr[ridx:ridx + 1, :], rrow)
                    rbc = rc_pool.tile([DH, 512], F32, tag="rbc", bufs=1,
                                       name=f"rb{h}_{n}")
                    bcast(rbc, rec_scr[ridx:ridx + 1, :])
                    nc.vector.tensor_mul(
                        oT[h // 2][(h % 2) * DH:(h % 2) * DH + DH,
                                   n * 512:(n + 1) * 512],
                        osl[0:DH, :], rbc)

        # GPB1 fold: x += G1*proj_b runs on GpSimd during attention
        for i in range(NT):
            nc.gpsimd.tensor_add(sx[i], sx[i], GPB1)

        # ---- proj (swapped: token-major out) + residual 1 (in-place x) ----
        for i in range(NT):
            ps = psum.tile([P, 1024], F32, tag="sg", name=f"prps{i}")
            for k in range(KC):
                nc.tensor.matmul(ps[:, 0:C], oT[k][:, i * P:(i + 1) * P],
                                 proj_sb[k], start=(k == 0), stop=(k == KC - 1))
            attn_sb = work.tile([P, C], BF16, tag="attnsb", bufs=2,
                                name=f"attnsb{i}")
            nc.scalar.copy(attn_sb, ps[:, 0:C])
            ta = work.tile([P, C], F32, tag="tmp", bufs=3, name=f"res1_{i}")
            nc.gpsimd.tensor_mul(ta, attn_sb, G1)
            nc.vector.tensor_add(sx[i], sx[i], ta)

        # ---- LN2 + modulate + transpose (h2T reuses h1T slots) ----
        h2T = [bigT.tile([P, T], BF16, tag="bigT", name=f"h2T{j}") for j in range(KC)]
        rstds2, negmrs2 = ln_stats_all("b")
        for i in range(NT):
            ln_apply(sx[i], i, rstds2[i], negmrs2[i], W2, B2, h2T, "b")
        # GPB2 fold after LN2 has consumed x2
        for i in range(NT):
            nc.gpsimd.tensor_add(sx[i], sx[i], GPB2)

        # ---- MLP per t-chunk; fc2 swapped -> token-major; residual 2 ----
        for n in range(NQ):
            fps = [psum.tile([P, 1024], F32, tag="oaccp", name=f"fps{n}_{sp}")
                   for sp in range(2)]

            def fc2_mms(m, g1t):
                for s in range(4):
                    nc.tensor.matmul(fps[s // 2][:, (s % 2) * 512:(s % 2) * 512 + 512],
                                     g1t[:, s * P:(s + 1) * P], fc2_sb[m],
                                     start=(m == 0), stop=(m == MLP // P - 1))

            g1_prev = None
            for m in range(MLP // P):
                ps = psum.tile([P, 1024], F32, tag="sg", name=f"f1ps{n}_{m}")
                for k in range(KC):
                    nc.tensor.matmul(ps[:, 0:C], fc1_sb[k][:, m * P:(m + 1) * P],
                                     h2T[k][:, n * 512:(n + 1) * 512],
                                     start=(k == 0), stop=(k == KC - 1))
                if g1_prev is not None:
                    fc2_mms(m - 1, g1_prev)
                g1 = work.tile([P, C], BF16, tag="g1", bufs=3, name=f"g1_{n}_{m}")
                nc.scalar.activation(g1, ps[:, 0:C], GELU_AF,
                                     bias=fc1b_sb[:, m:m + 1])
                g1_prev = g1
            fc2_mms(MLP // P - 1, g1_prev)
            for s in range(4):
                i = n * 4 + s
                mlp_sb = work.tile([P, C], BF16, tag="attnsb", bufs=2,
                                   name=f"mlpsb{i}")
                nc.scalar.copy(mlp_sb, fps[s // 2][:, (s % 2) * 512:(s % 2) * 512 + 512])
                tb = work.tile([P, C], F32, tag="tmp", bufs=3, name=f"res2_{i}")
                nc.gpsimd.tensor_mul(tb, mlp_sb, G2)
                nc.vector.tensor_add(sx[i], sx[i], tb)
                nc.sync.dma_start(out_d[i], sx[i])

    nc.compile()
    return nc


def make_in_maps(inputs):
    bf = ml_dtypes.bfloat16
    f32 = np.float32
    x = np.asarray(inputs["x"], f32)
    c = np.asarray(inputs["c"], f32)
    qkv_w = np.asarray(inputs["qkv_w"], f32)
    qkv_b = np.asarray(inputs["qkv_b"], f32)
    proj_w = np.asarray(inputs["proj_w"], f32)
    proj_b = np.asarray(inputs["proj_b"], f32)
    ada_w = np.asarray(inputs["ada_w"], f32)
    ada_b = np.asarray(inputs["ada_b"], f32)
    fc1_w = np.asarray(inputs["fc1_w"], f32)
    fc1_b = np.asarray(inputs["fc1_b"], f32)
    fc2_w = np.asarray(inputs["fc2_w"], f32)
    fc2_b = np.asarray(inputs["fc2_b"], f32)
    ln = {k: np.asarray(inputs[k], f32) for k in
          ["ln1_w", "ln1_b", "ln2_w", "ln2_b"]}

    shared = {
        "ada_wt": np.ascontiguousarray(ada_w.T.reshape(KC, P, 6 * C)).astype(bf),
        "qkv_wt": np.ascontiguousarray(qkv_w.T.reshape(KC, P, 3 * C)).astype(bf),
        "proj_wt": np.ascontiguousarray(proj_w.T.reshape(KC, P, C)).astype(bf),
        "fc1_wt": np.ascontiguousarray(fc1_w.T.reshape(KC, P, MLP)).astype(bf),
        "fc2_wt": np.ascontiguousarray(fc2_w.T.reshape(MLP // P, P, C)).astype(bf),
        "qkv_b_qk": np.ascontiguousarray(qkv_b[:2 * C].reshape(8, P).T).astype(f32),
        "fc1_b_c": np.ascontiguousarray(fc1_b.reshape(MLP // P, P).T).astype(f32),
        "vb_row": qkv_b[2 * C:].reshape(1, C).astype(f32),
    }
    # host-folded constant rows (weights-only algebra; inputs never touched):
    #   W = ln_w*(1+mod_sc) where mod_sc = dev_sc + ada_b_sc
    #     = dev_sc*A + D with A = ln_w, D = ln_w*(1+ada_b_sc); similarly B, G.
    for br, (lnw, lnb, pb) in {1: (ln["ln1_w"], ln["ln1_b"], proj_b),
                               2: (ln["ln2_w"], ln["ln2_b"], fc2_b)}.items():
        o = (br - 1) * 3 * C
        sh_ab = ada_b[o:o + C]
        sc_ab = ada_b[o + C:o + 2 * C]
        g_ab = ada_b[o + 2 * C:o + 3 * C]
        shared[f"A{br}"] = lnw.reshape(1, C).astype(f32)
        shared[f"D{br}"] = (lnw * (1 + sc_ab)).reshape(1, C).astype(f32)
        shared[f"A2{br}"] = lnb.reshape(1, C).astype(f32)
        shared[f"E{br}"] = (lnb * (1 + sc_ab) + sh_ab).reshape(1, C).astype(f32)
        shared[f"pb{br}"] = pb.reshape(1, C).astype(f32)
        shared[f"gb{br}"] = g_ab.reshape(1, C).astype(f32)
    maps = []
    for b in range(B):
        m = dict(shared)
        m["x"] = np.ascontiguousarray(x[b].reshape(NT, P, C))
        m["c_col"] = np.ascontiguousarray(c[b].reshape(KC, P).T)
        maps.append(m)
    return maps


_CACHED_NC = None


def run(inputs, trace=False):
    global _CACHED_NC
    if _CACHED_NC is None:
        _CACHED_NC = build_program()
    maps = make_in_maps(inputs)
    res = run_bass_kernel_spmd(_CACHED_NC, maps, core_ids=list(range(B)),
                               trace=trace)
    out = np.stack([res.results[b]["out"].reshape(T, C) for b in range(B)])
    return out.astype(np.float32), res


def kernel(**inputs) -> np.ndarray:
    out, _ = run(inputs, trace=False)
    return out
